# revision 29
# baseline (speedup 1.0000x reference)
"""Trainium2 Bass kernel for the GNN message-passing autoencoder problem.

Strategy (8 NeuronCores, SPMD), v2 (fp8):
  - Nodes sharded 1024/core. Message passing is a dense matmul against the
    PLAIN adjacency transpose shard A^T[:, shard] in fp8 e4m3 (counts are
    exact in fp8) using DoubleRow perf mode. GraphConv 'both' norms are
    folded into per-node scalings: D_src^-1/2 is applied to the (h @ W)
    activations (exact per-partition scale), D_dst^-1/2 multiplies the
    aggregation PSUM before bias+PReLU.
  - The per-layer linear W is applied BEFORE the AllGather (z = A (h W) ==
    (A h) W): lhsT = feature-major BN'd h, rhs = W, giving node-major
    activations p directly - no PE transposes in the layer loop. p is
    quantized to fp8 and AllGathered (4 MB full graph).
  - Layer epilogue: bias+PReLU fused in the PSUM eviction (scalar engine),
    BN stats partials AllReduced (4 KB), BN+PReLU fused in one activation.
  - The two chains are interleaved with a half-layer stagger so ARs/AGs hide
    under the other chain's matmuls.
  - Tail: loss2*N^2 = sum(M^2) - 2*tr(H^T M H) + ||H^T H||_F^2 with
    H = l2-normalized h2 in fp8. sum(M^2) on host; tr term via an fp8
    DoubleRow matmul (M^T shard stationary, gathered H moving) with a fused
    multiply-accumulate eviction; G = H^T H computed redundantly per core.
    loss1 (cosine^3) is computed per-shard in feature-major layout using
    ones-vector matmuls for the partition reductions.
"""

import os
import sys

for _p in ("/opt/trn_rl_repo", "/opt/pypackages"):
    if _p not in sys.path:
        sys.path.append(_p)

import numpy as np
import ml_dtypes

import concourse.bass as bass
import concourse.mybir as mybir
import concourse.tile as tile
from concourse import bacc
from concourse.bass_utils import run_bass_kernel_spmd
from concourse.masks import make_identity

F8 = mybir.dt.float8e4
BF16 = mybir.dt.bfloat16
F32 = mybir.dt.float32
AF = mybir.ActivationFunctionType
ALU = mybir.AluOpType
AX = mybir.AxisListType
DR = mybir.MatmulPerfMode.DoubleRow

N = 8192
F = 512
NCORES = 8
SH = N // NCORES          # 1024 nodes per core shard
NB = N // 128             # 64 node k-subtiles
SB = SH // 128            # 8 node blocks per shard
FB = F // 128             # 4 feature blocks
GROUPS = [list(range(NCORES))]

# layer-instance parameter rows: enc0 enc1 dec1_0 dec1_1 dec2_0 dec2_1
LI = {1: [0, 1, 2, 3], 2: [0, 1, 4, 5]}
# W row applied at the END of layer l (producing p for layer l+1)
WNEXT = {1: [1, 2, 3, None], 2: [1, 4, 5, None]}


def _emit_pf_load(nc, g, c, l):
    """Load the full-graph node-major fp8 activations for layer l."""
    sb = g["sb"]
    pf = sb.tile([128, NB, F], F8, tag=f"pf{c}", bufs=1, name="pf")
    if l == 0:
        src = g["p0"][c]
        for q in range(8):
            nc.sync.dma_start(pf[:, 8 * q:8 * q + 8, :],
                              src[:, 8 * q:8 * q + 8, :])
    else:
        src = g["agp_out"][(c, l - 1)]
        for cc in range(NCORES):
            nc.sync.dma_start(pf[:, 8 * cc:8 * cc + 8, :],
                              src[cc * 128:(cc + 1) * 128, :, :])
    g["pf"][c] = pf


def _emit_A_half(nc, g, c, l, half):
    """A-aggregation matmuls for one 512-dest half; evict with bias+PReLU."""
    sb, ps = g["sb"], g["ps"]
    li = LI[c][l]
    pf = g["pf"][c]
    a_dram = g["a_dram"][c]
    if half == 0:
        zt = sb.tile([128, FB, SH], BF16, tag=f"zt{c}", bufs=1, name="zt")
        g["zt"][c] = zt
    else:
        zt = g["zt"][c]
    zps = [ps.tile([128, 512], F32, tag=f"ps{c}", bufs=4, name="zps")
           for _ in range(FB)]
    for th in range(16):
        art = sb.tile([128, 4, 512], F8, tag=f"a{c}", bufs=4, name="art")
        nc.sync.dma_start(art[:], a_dram[half, :, 4 * th:4 * th + 4, :])
        for j in range(2):
            kp = 2 * th + j
            kk = 4 * th + 2 * j
            for m in range(FB):
                nc.tensor.matmul(
                    zps[m][:],
                    pf[:, kk:kk + 2, m * 128:(m + 1) * 128],
                    art[:, 2 * j:2 * j + 2, :],
                    start=(kp == 0), stop=(kp == 31), perf_mode=DR)
    for m in range(FB):
        dst = zt[:, m, half * 512:(half + 1) * 512]
        bias = g["b_sb"][:, li, m:m + 1]
        alpha = g["al_sb"][:, 2 * li:2 * li + 1]
        if l < 2:  # enc layer: multiply by ddst before bias+prelu
            zsc = sb.tile([128, 512], F32, tag="scrh", bufs=4, name="zsc")
            nc.vector.tensor_tensor(
                zsc[:], zps[m][:],
                g["ddb"][c][:, half * 512:(half + 1) * 512], ALU.mult)
            nc.scalar.activation(dst, zsc[:], AF.Prelu, bias=bias, scale=1.0,
                                 alpha=alpha)
        else:
            nc.scalar.activation(dst, zps[m][:], AF.Prelu, bias=bias,
                                 scale=1.0, alpha=alpha)


def _emit_stats_ar(nc, g, c, l):
    """Per-core BN stats (sum, sumsq per feature) and the AllReduce."""
    sb = g["sb"]
    zt = g["zt"][c]
    stats = sb.tile([128, 8], F32, tag=f"st{c}", bufs=1, name="stats")
    for m in range(FB):
        nc.vector.reduce_sum(stats[:, 2 * m:2 * m + 1], zt[:, m, :], axis=AX.X)
        scr = sb.tile([128, SH], F32, tag="scr", bufs=1, name="scr")
        nc.scalar.activation(scr[:], zt[:, m, :], AF.Square,
                             accum_out=stats[:, 2 * m + 1:2 * m + 2])
    ar_in = g["ar_in"][(c, l)]
    ar_out = g["ar_out"][(c, l)]
    nc.sync.dma_start(ar_in[:], stats[:])
    nc.gpsimd.collective_compute(
        "AllReduce", ALU.add, replica_groups=GROUPS,
        ins=[ar_in[:]], outs=[ar_out[:]])


def _emit_bn_apply(nc, g, c, l):
    """BN finalize from the AllReduced stats; fused BN+PReLU in place."""
    sb = g["sb"]
    li = LI[c][l]
    zt = g["zt"][c]
    gstats = sb.tile([128, 8], F32, tag="gstats", name="gstats")
    nc.sync.dma_start(gstats[:], g["ar_out"][(c, l)][:])
    mean = sb.tile([128, FB], F32, tag="mean", name="mean")
    var = sb.tile([128, FB], F32, tag="var", name="var")
    sN = sb.tile([128, FB], F32, tag="sN", name="sN")
    tN = sb.tile([128, FB], F32, tag="tN", name="tN")
    m2 = sb.tile([128, FB], F32, tag="m2", name="m2")
    nc.scalar.mul(mean[:], gstats[:, 0:8:2], 1.0 / N)
    nc.scalar.mul(var[:], gstats[:, 1:8:2], 1.0 / N)      # E[x^2]
    nc.vector.tensor_mul(m2[:], mean[:], mean[:])
    nc.vector.tensor_sub(var[:], var[:], m2[:])
    nc.scalar.activation(sN[:], var[:], AF.Sqrt, bias=g["epsb"][:])
    nc.vector.reciprocal(sN[:], sN[:])
    nc.vector.tensor_mul(sN[:], sN[:], g["g_sb"][:, li, :])
    nc.vector.tensor_mul(m2[:], mean[:], sN[:])
    nc.vector.tensor_sub(tN[:], g["bb_sb"][:, li, :], m2[:])
    for m in range(FB):
        nc.scalar.activation(
            zt[:, m, :], zt[:, m, :], AF.Prelu,
            bias=tN[:, m:m + 1], scale=sN[:, m:m + 1],
            alpha=g["al_sb"][:, 2 * li + 1:2 * li + 2])


def _emit_w_ag(nc, g, c, l):
    """p = (BN'd h) @ W_next (node-major out), fp8 quantize, AllGather."""
    sb, ps = g["sb"], g["ps"]
    zt = g["zt"][c]
    li_w = WNEXT[c][l]
    p_out = sb.tile([128, SB, F], F8, tag=f"po{c}", bufs=2, name="p_out")
    for wave in range(2):
        pps = [ps.tile([128, 512], F32, tag=f"ps{c}", bufs=4, name="pps")
               for _ in range(4)]
        for i in range(4):
            tb = wave * 4 + i
            for kb in range(FB):
                nc.tensor.matmul(
                    pps[i][:], zt[:, kb, tb * 128:(tb + 1) * 128],
                    g["w_sb"][:, li_w - 1, kb, :],
                    start=(kb == 0), stop=(kb == FB - 1))
        for i in range(4):
            tb = wave * 4 + i
            if l == 0:  # next layer is enc (normalized): scale by dsrc
                nc.vector.tensor_scalar_mul(
                    p_out[:, tb, :], pps[i][:], g["dsc"][c][:, tb:tb + 1])
            else:
                nc.vector.tensor_copy(p_out[:, tb, :], pps[i][:])
    agp_in = g["agp_in"][(c, l)]
    nc.sync.dma_start(agp_in[:], p_out[:])
    nc.gpsimd.collective_compute(
        "AllGather", ALU.bypass, replica_groups=GROUPS,
        ins=[agp_in[:]], outs=[g["agp_out"][(c, l)][:]])
    if (c, l) in g["dbg"]:
        nc.sync.dma_start(g["dbg"][(c, l)][:], p_out[:])


def _emit_tail1(nc, g):
    """Chain1 tail: loss1 partial = sum over shard of (1 - cos(h1, attr))^3."""
    sb, ps = g["sb"], g["ps"]
    zt = g["zt"][1]
    attrTn = g["attrTn"]
    dotps = [ps.tile([1, 512], F32, tag="ps1", bufs=4, name="dotps")
             for _ in range(2)]
    n1ps = [ps.tile([1, 512], F32, tag="ps1", bufs=4, name="n1ps")
            for _ in range(2)]
    for kb in range(FB):
        prod = sb.tile([128, SH], BF16, tag="prod", bufs=2, name="prod")
        nc.vector.tensor_tensor(prod[:], zt[:, kb, :], attrTn[:, kb, :],
                                ALU.mult)
        sq = sb.tile([128, SH], BF16, tag="sqh", bufs=2, name="sq")
        nc.gpsimd.tensor_tensor(sq[:], zt[:, kb, :], zt[:, kb, :], ALU.mult)
        for h in range(2):
            nc.tensor.matmul(dotps[h][:], g["ones128"][:],
                             prod[:, h * 512:(h + 1) * 512],
                             start=(kb == 0), stop=(kb == FB - 1))
            nc.tensor.matmul(n1ps[h][:], g["ones128"][:],
                             sq[:, h * 512:(h + 1) * 512],
                             start=(kb == 0), stop=(kb == FB - 1))
    dot_sb = sb.tile([1, SH], F32, tag="row", bufs=3, name="dot_sb")
    n1_sb = sb.tile([1, SH], F32, tag="row", bufs=3, name="n1_sb")
    for h in range(2):
        nc.vector.tensor_copy(dot_sb[:, h * 512:(h + 1) * 512], dotps[h][:])
        nc.vector.tensor_copy(n1_sb[:, h * 512:(h + 1) * 512], n1ps[h][:])
    nc.scalar.activation(n1_sb[:], n1_sb[:], AF.Sqrt)
    nc.vector.tensor_scalar_max(n1_sb[:], n1_sb[:], 1e-12)
    nc.vector.reciprocal(n1_sb[:], n1_sb[:])
    nc.vector.tensor_mul(dot_sb[:], dot_sb[:], n1_sb[:])        # cos
    u = sb.tile([1, SH], F32, tag="row", bufs=3, name="u")
    nc.scalar.activation(u[:], dot_sb[:], AF.Copy, scale=-1.0, bias=1.0)
    u2 = sb.tile([1, SH], F32, tag="row", bufs=3, name="u2")
    nc.vector.tensor_mul(u2[:], u[:], u[:])
    nc.vector.tensor_mul(u2[:], u2[:], u[:])                    # u^3
    l1p = sb.tile([1, 1], F32, tag="l1p", name="l1p")
    nc.vector.reduce_sum(l1p[:], u2[:], axis=AX.X)
    g["l1p"] = l1p


def _emit_tail2(nc, g):
    """Chain2 tail: l2-normalize h2 (via PE transposes), fp8, AllGather."""
    sb, ps = g["sb"], g["ps"]
    zt = g["zt"][2]
    h2nm = sb.tile([128, SB, F], BF16, tag="h2nm", bufs=1, name="h2nm")
    nrm2 = sb.tile([128, SB], F32, tag="nrm2", bufs=1, name="nrm2")
    for tb in range(SB):
        for m in range(FB):
            tp = ps.tile([128, 128], BF16, tag="ps2", bufs=4, name="tp")
            nc.tensor.transpose(tp[:], zt[:, m, tb * 128:(tb + 1) * 128],
                                g["ident"][:])
            nc.vector.tensor_copy(h2nm[:, tb, m * 128:(m + 1) * 128], tp[:])
        scr = sb.tile([128, 512], F32, tag="scrh", bufs=4, name="sqs")
        nc.scalar.activation(scr[:], h2nm[:, tb, :], AF.Square,
                             accum_out=nrm2[:, tb:tb + 1])
    nc.scalar.activation(nrm2[:], nrm2[:], AF.Sqrt)
    nc.vector.tensor_scalar_max(nrm2[:], nrm2[:], 1e-12)
    nc.vector.reciprocal(nrm2[:], nrm2[:])
    h2q = sb.tile([128, SB, F], F8, tag="h2q", bufs=1, name="h2q")
    for tb in range(SB):
        nc.vector.tensor_scalar_mul(h2q[:, tb, :], h2nm[:, tb, :],
                                    nrm2[:, tb:tb + 1])
    nc.sync.dma_start(g["ag2_in"][:], h2q[:])
    nc.gpsimd.collective_compute(
        "AllGather", ALU.bypass, replica_groups=GROUPS,
        ins=[g["ag2_in"][:]], outs=[g["ag2_out"][:]])
    g["h2q"] = h2q
    # local partial of G = H^T H (k over the local 8 node blocks), then
    # AllReduce it while the MH matmul runs
    gq = [ps.tile([128, 512], F32, tag="ps1", bufs=4, name="gq")
          for _ in range(FB)]
    for t in range(SB // 2):
        for mb in range(FB):
            nc.tensor.matmul(
                gq[mb][:], h2q[:, 2 * t:2 * t + 2, mb * 128:(mb + 1) * 128],
                h2q[:, 2 * t:2 * t + 2, :],
                start=(t == 0), stop=(t == SB // 2 - 1), perf_mode=DR)
    for mb in range(FB):
        gsc = sb.tile([128, 512], F32, tag="scrh", bufs=4, name="gsc")
        nc.vector.tensor_copy(gsc[:], gq[mb][:])
        nc.sync.dma_start(g["arg_in"][:, mb * 512:(mb + 1) * 512], gsc[:])
    nc.gpsimd.collective_compute(
        "AllReduce", ALU.add, replica_groups=GROUPS,
        ins=[g["arg_in"][:]], outs=[g["arg_out"][:]])
    if "h2q" in g["dbg"]:
        nc.sync.dma_start(g["dbg"]["h2q"][:], h2q[:])


def _emit_tail_mh_g(nc, g):
    """MH = M @ H (fp8 DR) with fused (MH*H) accumulation; partials."""
    sb, ps = g["sb"], g["ps"]
    # prefetch the first M^T chunks before the H reload
    mtcs = {}
    for t in range(2):
        mtc = sb.tile([128, 2, SH], F8, tag="mtc", bufs=3, name="mtc")
        nc.sync.dma_start(mtc[:], g["mt_dram"][:, 2 * t:2 * t + 2, :])
        mtcs[t] = mtc
    hf = sb.tile([128, NB, F], F8, tag="pf2", bufs=1, name="hf")
    for cc in range(NCORES):
        nc.sync.dma_start(hf[:, 8 * cc:8 * cc + 8, :],
                          g["ag2_out"][cc * 128:(cc + 1) * 128, :, :])
    mhps = [ps.tile([128, 512], F32, tag="ps1", bufs=4, name="mhps")
            for _ in range(4)]
    mhps += [ps.tile([128, 512], F32, tag="ps2", bufs=4, name="mhps2")
             for _ in range(4)]
    for t in range(32):
        if t in mtcs:
            mtc = mtcs[t]
        else:
            mtc = sb.tile([128, 2, SH], F8, tag="mtc", bufs=3, name="mtc")
            nc.sync.dma_start(mtc[:], g["mt_dram"][:, 2 * t:2 * t + 2, :])
        for ib in range(8):
            nc.tensor.matmul(
                mhps[ib][:], mtc[:, :, ib * 128:(ib + 1) * 128],
                hf[:, 2 * t:2 * t + 2, :],
                start=(t == 0), stop=(t == 31), perf_mode=DR)
    xacc = sb.tile([128, 8], F32, tag="xacc", bufs=1, name="xacc")
    h2q = g["h2q"]
    for ib in range(8):
        scr = sb.tile([128, 512], F32, tag="scrh", bufs=4, name="xscr")
        nc.vector.scalar_tensor_tensor(
            scr[:], mhps[ib][:], 1.0, h2q[:, ib, :], op0=ALU.mult,
            op1=ALU.mult, accum_out=xacc[:, ib:ib + 1])
    # gsq from the AllReduced G partials (reuses the h2nm slot)
    gsb = sb.tile([128, FB, 512], F32, tag="h2nm", bufs=1, name="gsb")
    nc.sync.dma_start(gsb[:], g["arg_out"].rearrange("p (m f) -> p m f", m=FB))
    gacc = sb.tile([128, FB], F32, tag="gacc", bufs=1, name="gacc")
    for mb in range(FB):
        scr = sb.tile([128, 512], F32, tag="scrh", bufs=4, name="gscr")
        nc.scalar.activation(scr[:], gsb[:, mb, :], AF.Square,
                             accum_out=gacc[:, mb:mb + 1])
    g["xacc"] = xacc
    g["gacc"] = gacc


def _emit_partials(nc, g):
    sb, ps = g["sb"], g["ps"]
    # combine partials: [l1p_sum, cross_sum, gsq_sum]
    pl = sb.tile([128, 3], F32, tag="pl", name="pl")
    nc.vector.memset(pl[:], 0.0)
    if "l1p" in g:
        nc.vector.tensor_copy(pl[0:1, 0:1], g["l1p"][:])
    if "xacc" in g:
        nc.vector.reduce_sum(pl[:, 1:2], g["xacc"][:], axis=AX.X)
        nc.vector.reduce_sum(pl[:, 2:3], g["gacc"][:], axis=AX.X)
    pp = ps.tile([3, 1], F32, tag="ps1", bufs=4, name="pp")
    nc.tensor.matmul(pp[:], pl[:], g["onesf"][:], start=True, stop=True)
    out_sb = sb.tile([3, 1], F32, tag="out_sb", name="out_sb")
    nc.scalar.copy(out_sb[:], pp[:])
    nc.sync.dma_start(g["partials"][:], out_sb[:])


STOP_LV = {"l0h": 0, "l0s": 0.3, "l0w": 0.6, "l0": 1, "layers": 2, "tail1": 3,
           "tail2": 4, "full": 5}


def build_nc():
    nc = bacc.Bacc("TRN2", target_bir_lowering=False, debug=False,
                   num_devices=NCORES)

    ins = {}

    def di(name, shape, dt):
        ins[name] = nc.dram_tensor(name, shape, dt, kind="ExternalInput")
        return ins[name]

    p1_0 = di("p1_0", [128, NB, F], F8)
    p2_0 = di("p2_0", [128, NB, F], F8)
    a1 = di("a1", [2, 128, NB, 512], F8)
    a2 = di("a2", [2, 128, NB, 512], F8)
    mt = di("mt", [128, NB, SH], F8)
    w_all = di("w_all", [6, 128, FB, F], BF16)
    b_all = di("b_all", [6, F], F32)
    g_all = di("g_all", [6, F], F32)
    bb_all = di("bb_all", [6, F], F32)
    al_all = di("al_all", [1, 12], F32)
    ddb1 = di("ddb1", [128, SH], F32)
    ddb2 = di("ddb2", [128, SH], F32)
    dsc1 = di("dsc1", [128, SB], F32)
    dsc2 = di("dsc2", [128, SB], F32)
    attrTn = di("attrTn", [128, FB, SH], BF16)

    partials = nc.dram_tensor("partials", [3, 1], F32, kind="ExternalOutput")

    ar_in, ar_out, agp_in, agp_out = {}, {}, {}, {}
    for c in (1, 2):
        for l in range(4):
            ar_in[(c, l)] = nc.dram_tensor(f"ar_in_{c}_{l}", [128, 8], F32)
            ar_out[(c, l)] = nc.dram_tensor(f"ar_out_{c}_{l}", [128, 8], F32,
                                            addr_space="Shared")
            if l < 3:
                agp_in[(c, l)] = nc.dram_tensor(f"agp_in_{c}_{l}",
                                                [128, SB, F], F8)
                agp_out[(c, l)] = nc.dram_tensor(f"agp_out_{c}_{l}",
                                                 [NCORES * 128, SB, F], F8,
                                                 addr_space="Shared")
    ag2_in = nc.dram_tensor("ag2_in", [128, SB, F], F8)
    ag2_out = nc.dram_tensor("ag2_out", [NCORES * 128, SB, F], F8,
                             addr_space="Shared")
    warm_in = nc.dram_tensor("warm_in", [128, 8], F32)
    warm_out = nc.dram_tensor("warm_out", [128, 8], F32, addr_space="Shared")
    arg_in = nc.dram_tensor("arg_in", [128, FB * 512], F32)
    arg_out = nc.dram_tensor("arg_out", [128, FB * 512], F32,
                             addr_space="Shared")

    dbg = {}
    if os.environ.get("BASSK_DEBUG"):
        for c in (1, 2):
            for l in range(3):
                dbg[(c, l)] = nc.dram_tensor(f"dbg_p_{c}_{l}", [128, SB, F],
                                             F8, kind="ExternalOutput")
        dbg["h2q"] = nc.dram_tensor("dbg_h2q", [128, SB, F], F8,
                                    kind="ExternalOutput")

    with tile.TileContext(nc) as tc:
        with (
            tc.tile_pool(name="sb", bufs=2) as sb,
            tc.tile_pool(name="ps", bufs=4, space="PSUM") as ps,
        ):
            g = {
                "sb": sb, "ps": ps, "dbg": dbg, "partials": partials,
                "a_dram": {1: a1, 2: a2}, "mt_dram": mt,
                "p0": {1: p1_0, 2: p2_0},
                "ar_in": ar_in, "ar_out": ar_out,
                "agp_in": agp_in, "agp_out": agp_out,
                "ag2_in": ag2_in, "ag2_out": ag2_out,
                "pf": {}, "zt": {},
                "warm_in": warm_in, "warm_out": warm_out,
                "arg_in": arg_in, "arg_out": arg_out,
            }
            # ---- constants / params ----
            ident = sb.tile([128, 128], BF16, tag="ident", bufs=1, name="ident")
            make_identity(nc, ident[:])
            g["ident"] = ident
            for nm, src in (("b_sb", b_all), ("g_sb", g_all), ("bb_sb", bb_all)):
                t = sb.tile([128, 6, FB], F32, tag=nm, bufs=1, name=nm)
                nc.sync.dma_start(t[:], src.rearrange("l (m p) -> p l m", p=128))
                g[nm] = t
            al1 = sb.tile([1, 12], F32, tag="al1", bufs=1, name="al1")
            nc.sync.dma_start(al1[:], al_all[:])
            al_sb = sb.tile([128, 12], F32, tag="al_sb", bufs=1, name="al_sb")
            nc.gpsimd.partition_broadcast(al_sb[:], al1[:])
            g["al_sb"] = al_sb
            epsb = sb.tile([128, 1], F32, tag="epsb", bufs=1, name="epsb")
            nc.vector.memset(epsb[:], 1e-5)
            g["epsb"] = epsb
            ones128 = sb.tile([128, 1], BF16, tag="ones128", bufs=1, name="ones128")
            nc.vector.memset(ones128[:], 1.0)
            g["ones128"] = ones128
            onesf = sb.tile([128, 1], F32, tag="onesf", bufs=1, name="onesf")
            nc.vector.memset(onesf[:], 1.0)
            g["onesf"] = onesf
            # warm up the collective path during the startup loads
            if os.environ.get("BASSK_WARM"):
                # no measured benefit: early-collective latency jitter persists
                wsb = sb.tile([128, 8], F32, tag="wsb", bufs=1, name="wsb")
                nc.vector.memset(wsb[:], 0.0)
                nc.sync.dma_start(g["warm_in"][:], wsb[:])
                nc.gpsimd.collective_compute(
                    "AllReduce", ALU.add, replica_groups=GROUPS,
                    ins=[g["warm_in"][:]], outs=[g["warm_out"][:]])

            # ---- staggered 2-chain layer pipeline ----
            EM = STOP_LV[os.environ.get("BASSK_STOP", "full")]
            _emit_pf_load(nc, g, 1, 0)
            # ddb1 is needed by the first eviction; the rest after the A DMAs
            ddbt = {}
            t1 = sb.tile([128, SH], F32, tag="ddb1", bufs=1, name="ddb")
            nc.sync.dma_start(t1[:], ddb1[:])
            ddbt[1] = t1
            g["ddb"] = ddbt
            _emit_A_half(nc, g, 1, 0, 0)
            t2 = sb.tile([128, SH], F32, tag="ddb2", bufs=1, name="ddb")
            nc.sync.dma_start(t2[:], ddb2[:])
            ddbt[2] = t2
            dsct = {}
            for c, src in ((1, dsc1), (2, dsc2)):
                t = sb.tile([128, SB], F32, tag=f"dsc{c}", bufs=1, name="dsc")
                nc.sync.dma_start(t[:], src[:])
                dsct[c] = t
            g["dsc"] = dsct
            w_sb = sb.tile([128, 5, FB, F], BF16, tag="w_sb", bufs=1, name="w_sb")
            for li in (1, 2, 3, 4, 5):  # row 0 (enc0) applied on host
                nc.sync.dma_start(w_sb[:, li - 1], w_all[li])
            g["w_sb"] = w_sb
            at = sb.tile([128, FB, SH], BF16, tag="attrTn", bufs=1, name="attrTn")
            nc.sync.dma_start(at[:], attrTn[:])
            g["attrTn"] = at
            if EM >= 0.3:
                _emit_A_half(nc, g, 1, 0, 1)
                _emit_stats_ar(nc, g, 1, 0)
            if EM >= 0.6:
                _emit_pf_load(nc, g, 2, 0)
                _emit_A_half(nc, g, 2, 0, 0)
                _emit_bn_apply(nc, g, 1, 0)
                _emit_w_ag(nc, g, 1, 0)
            if EM >= 1:
                _emit_A_half(nc, g, 2, 0, 1)
                _emit_stats_ar(nc, g, 2, 0)
            if EM >= 2:
                for l in range(1, 4):
                    _emit_pf_load(nc, g, 1, l)
                    _emit_A_half(nc, g, 1, l, 0)
                    _emit_bn_apply(nc, g, 2, l - 1)
                    _emit_w_ag(nc, g, 2, l - 1)
                    _emit_A_half(nc, g, 1, l, 1)
                    _emit_stats_ar(nc, g, 1, l)
                    _emit_pf_load(nc, g, 2, l)
                    _emit_A_half(nc, g, 2, l, 0)
                    if l < 3:
                        _emit_bn_apply(nc, g, 1, l)
                        _emit_w_ag(nc, g, 1, l)
                        _emit_A_half(nc, g, 2, l, 1)
                        _emit_stats_ar(nc, g, 2, l)
                    else:
                        _emit_A_half(nc, g, 2, l, 1)
                        _emit_stats_ar(nc, g, 2, l)
                        _emit_bn_apply(nc, g, 1, l)
                        if EM >= 3:
                            _emit_tail1(nc, g)
            if EM >= 4:
                _emit_bn_apply(nc, g, 2, 3)
                _emit_tail2(nc, g)
            if EM >= 5:
                _emit_tail_mh_g(nc, g)
            _emit_partials(nc, g)

    nc.compile()
    return nc


_NC_CACHE = None


def _get_nc():
    global _NC_CACHE
    if _NC_CACHE is None:
        _NC_CACHE = build_nc()
    return _NC_CACHE


def _dinv(idx):
    deg = np.bincount(idx, minlength=N).astype(np.float32)
    return 1.0 / np.sqrt(np.clip(deg, 1.0, None))


def _adj_t(src, dst):
    """A^T[s, d] = multiplicity of edge s->d, float32 [N, N]."""
    flat = src.astype(np.int64) * N + dst.astype(np.int64)
    return np.bincount(flat, minlength=N * N).astype(np.float32).reshape(N, N)


def _swz_nodes(x, width):
    """[8192, width] -> [128, 64, width] with node = t*128 + p."""
    return np.ascontiguousarray(
        x.reshape(NB, 128, width).transpose(1, 0, 2))


def _swz_a(x):
    """[8192, 1024] -> [2, 128, 64, 512]: per dest-half, contiguous chunks."""
    sw = x.reshape(NB, 128, 2, 512).transpose(2, 0, 1, 3)
    return np.ascontiguousarray(sw.transpose(0, 2, 1, 3))


def host_prep(inputs):
    f8 = ml_dtypes.float8_e4m3
    bf16 = ml_dtypes.bfloat16
    attr = np.asarray(inputs["attr"], np.float32)
    matrix = np.asarray(inputs["matrix"], np.float32)
    mask1 = np.asarray(inputs["enc_mask_token1"], np.float32)
    src = np.asarray(inputs["src"]); dst = np.asarray(inputs["dst"])
    src2 = np.asarray(inputs["src2"]); dst2 = np.asarray(inputs["dst2"])
    tok = np.asarray(inputs["token_nodes"])
    noi = np.asarray(inputs["noise_nodes"])
    nsrc = np.asarray(inputs["noise_src"])

    x = attr.copy()
    x[tok] = 0.0
    x[noi] = attr[nsrc]
    np.add.at(x, tok, mask1[0])

    d1s, d1d = _dinv(src), _dinv(dst)
    d2s, d2d = _dinv(src2), _dinv(dst2)

    a1t = _adj_t(src, dst)    # A^T[s, d]
    a2t = _adj_t(src2, dst2)

    W0 = np.asarray(inputs["enc_W"][0], np.float32)
    p1_0 = _swz_nodes((d1s[:, None] * (x @ W0)).astype(f8), F)
    p2_0 = _swz_nodes((d2s[:, None] * (attr @ W0)).astype(f8), F)

    # w rows: enc0 enc1 d10 d11 d20 d21; device layout [6, 128, 4, 512]
    w_list = [np.asarray(inputs["enc_W"][0]), np.asarray(inputs["enc_W"][1]),
              np.asarray(inputs["dec1_W"][0]), np.asarray(inputs["dec1_W"][1]),
              np.asarray(inputs["dec2_W"][0]), np.asarray(inputs["dec2_W"][1])]
    w_all = np.ascontiguousarray(np.stack(
        [w.reshape(FB, 128, F).transpose(1, 0, 2) for w in w_list]
    ).astype(bf16))

    def stack6(key):
        return np.stack([
            np.asarray(inputs[f"enc_{key}"][0]), np.asarray(inputs[f"enc_{key}"][1]),
            np.asarray(inputs[f"dec1_{key}"][0]), np.asarray(inputs[f"dec1_{key}"][1]),
            np.asarray(inputs[f"dec2_{key}"][0]), np.asarray(inputs[f"dec2_{key}"][1]),
        ]).astype(np.float32)

    b_all, g_all, bb_all = stack6("b"), stack6("g"), stack6("bb")
    al = np.zeros((1, 12), np.float32)
    for i, (sa, so) in enumerate((("enc", 0), ("enc", 1), ("dec1", 0),
                                  ("dec1", 1), ("dec2", 0), ("dec2", 1))):
        al[0, 2 * i] = np.asarray(inputs[f"{sa}_ain"])[so]
        al[0, 2 * i + 1] = np.asarray(inputs[f"{sa}_aout"])[so]

    an = attr / np.maximum(np.linalg.norm(attr, axis=-1, keepdims=True), 1e-12)
    sumM2 = float(np.sum(matrix.astype(np.float64) ** 2))

    a1q = a1t.astype(f8)
    a2q = a2t.astype(f8)

    in_maps = []
    for c in range(NCORES):
        sl = slice(c * SH, (c + 1) * SH)
        # attrTn: feature-major [128, 4, 1024] for this shard
        at_sh = np.ascontiguousarray(
            an[sl].T.reshape(FB, 128, SH).transpose(1, 0, 2)).astype(bf16)
        in_maps.append({
            "p1_0": p1_0, "p2_0": p2_0,
            "a1": _swz_a(a1q[:, sl]),
            "a2": _swz_a(a2q[:, sl]),
            "mt": _swz_nodes(
                np.ascontiguousarray(matrix[sl].T).astype(f8), SH),
            "w_all": w_all, "b_all": b_all, "g_all": g_all, "bb_all": bb_all,
            "al_all": al,
            "ddb1": np.ascontiguousarray(
                np.broadcast_to(d1d[sl], (128, SH))).astype(np.float32),
            "ddb2": np.ascontiguousarray(
                np.broadcast_to(d2d[sl], (128, SH))).astype(np.float32),
            "dsc1": np.ascontiguousarray(d1s[sl].reshape(SB, 128).T),
            "dsc2": np.ascontiguousarray(d2s[sl].reshape(SB, 128).T),
            "attrTn": at_sh,
        })
    return in_maps, sumM2


def combine(results, sumM2):
    l1 = sum(float(r["partials"][0, 0]) for r in results)
    cross = sum(float(r["partials"][1, 0]) for r in results)
    gsq = np.mean([float(r["partials"][2, 0]) for r in results])
    loss1 = l1 / N
    loss2 = (sumM2 - 2.0 * cross + gsq) / (float(N) * N)
    return np.asarray(0.5 * loss1 + 0.5 * loss2, dtype=np.float32)


def run(inputs, trace=False, trace_kwargs=None):
    nc = _get_nc()
    in_maps, sumM2 = host_prep(inputs)
    res = run_bass_kernel_spmd(nc, in_maps, core_ids=list(range(NCORES)),
                               trace=trace, **(trace_kwargs or {}))
    return combine(res.results, sumM2), res


def kernel(**inputs) -> np.ndarray:
    out, _ = run(inputs, trace=False)
    return out


# revision 31
# speedup vs baseline: 1.0022x; 1.0022x over previous
"""Trainium2 Bass kernel for the GNN message-passing autoencoder problem.

Strategy (8 NeuronCores, SPMD), v2 (fp8):
  - Nodes sharded 1024/core. Message passing is a dense matmul against the
    PLAIN adjacency transpose shard A^T[:, shard] in fp8 e4m3 (counts are
    exact in fp8) using DoubleRow perf mode. GraphConv 'both' norms are
    folded into per-node scalings: D_src^-1/2 is applied to the (h @ W)
    activations (exact per-partition scale), D_dst^-1/2 multiplies the
    aggregation PSUM before bias+PReLU.
  - The per-layer linear W is applied BEFORE the AllGather (z = A (h W) ==
    (A h) W): lhsT = feature-major BN'd h, rhs = W, giving node-major
    activations p directly - no PE transposes in the layer loop. p is
    quantized to fp8 and AllGathered (4 MB full graph).
  - Layer epilogue: bias+PReLU fused in the PSUM eviction (scalar engine),
    BN stats partials AllReduced (4 KB), BN+PReLU fused in one activation.
  - The two chains are interleaved with a half-layer stagger so ARs/AGs hide
    under the other chain's matmuls.
  - Tail: loss2*N^2 = sum(M^2) - 2*tr(H^T M H) + ||H^T H||_F^2 with
    H = l2-normalized h2 in fp8. sum(M^2) on host; tr term via an fp8
    DoubleRow matmul (M^T shard stationary, gathered H moving) with a fused
    multiply-accumulate eviction; G = H^T H computed redundantly per core.
    loss1 (cosine^3) is computed per-shard in feature-major layout using
    ones-vector matmuls for the partition reductions.
"""

import os
import sys

for _p in ("/opt/trn_rl_repo", "/opt/pypackages"):
    if _p not in sys.path:
        sys.path.append(_p)

import numpy as np
import ml_dtypes

import concourse.bass as bass
import concourse.mybir as mybir
import concourse.tile as tile
from concourse import bacc
from concourse.bass_utils import run_bass_kernel_spmd
from concourse.masks import make_identity

F8 = mybir.dt.float8e4
BF16 = mybir.dt.bfloat16
F32 = mybir.dt.float32
AF = mybir.ActivationFunctionType
ALU = mybir.AluOpType
AX = mybir.AxisListType
DR = mybir.MatmulPerfMode.DoubleRow

N = 8192
F = 512
NCORES = 8
SH = N // NCORES          # 1024 nodes per core shard
NB = N // 128             # 64 node k-subtiles
SB = SH // 128            # 8 node blocks per shard
FB = F // 128             # 4 feature blocks
GROUPS = [list(range(NCORES))]

# layer-instance parameter rows: enc0 enc1 dec1_0 dec1_1 dec2_0 dec2_1
LI = {1: [0, 1, 2, 3], 2: [0, 1, 4, 5]}
# W row applied at the END of layer l (producing p for layer l+1)
WNEXT = {1: [1, 2, 3, None], 2: [1, 4, 5, None]}


def _emit_pf_load(nc, g, c, l):
    """Load the full-graph node-major fp8 activations for layer l."""
    sb = g["sb"]
    pf = sb.tile([128, NB, F], F8, tag=f"pf{c}", bufs=1, name="pf")
    if l == 0:
        src = g["p0"][c]
        for q in range(8):
            nc.sync.dma_start(pf[:, 8 * q:8 * q + 8, :],
                              src[:, 8 * q:8 * q + 8, :])
    else:
        src = g["agp_out"][(c, l - 1)]
        for cc in range(NCORES):
            nc.sync.dma_start(pf[:, 8 * cc:8 * cc + 8, :],
                              src[cc * 128:(cc + 1) * 128, :, :])
    g["pf"][c] = pf


def _emit_A_half(nc, g, c, l, half):
    """A-aggregation matmuls for one 512-dest half; evict with bias+PReLU."""
    sb, ps = g["sb"], g["ps"]
    li = LI[c][l]
    pf = g["pf"][c]
    a_dram = g["a_dram"][c]
    if half == 0:
        zt = sb.tile([128, FB, SH], BF16, tag=f"zt{c}", bufs=1, name="zt")
        g["zt"][c] = zt
    else:
        zt = g["zt"][c]
    zps = [ps.tile([128, 512], F32, tag=f"ps{c}", bufs=4, name="zps")
           for _ in range(FB)]
    for th in range(16):
        art = sb.tile([128, 4, 512], F8, tag=f"a{c}", bufs=3, name="art")
        nc.sync.dma_start(art[:], a_dram[half, :, 4 * th:4 * th + 4, :])
        for j in range(2):
            kp = 2 * th + j
            kk = 4 * th + 2 * j
            for m in range(FB):
                nc.tensor.matmul(
                    zps[m][:],
                    pf[:, kk:kk + 2, m * 128:(m + 1) * 128],
                    art[:, 2 * j:2 * j + 2, :],
                    start=(kp == 0), stop=(kp == 31), perf_mode=DR)
    for m in range(FB):
        dst = zt[:, m, half * 512:(half + 1) * 512]
        bias = g["b_sb"][:, li, m:m + 1]
        alpha = g["al_sb"][:, 2 * li:2 * li + 1]
        if l < 2:  # enc layer: multiply by ddst before bias+prelu
            zsc = sb.tile([128, 512], F32, tag="scrh", bufs=4, name="zsc")
            nc.vector.tensor_tensor(
                zsc[:], zps[m][:],
                g["ddb"][c][:, half * 512:(half + 1) * 512], ALU.mult)
            nc.scalar.activation(dst, zsc[:], AF.Prelu, bias=bias, scale=1.0,
                                 alpha=alpha)
        else:
            nc.scalar.activation(dst, zps[m][:], AF.Prelu, bias=bias,
                                 scale=1.0, alpha=alpha)


def _emit_stats_ar(nc, g, c, l):
    """Per-core BN stats (sum, sumsq per feature) and the AllReduce."""
    sb = g["sb"]
    zt = g["zt"][c]
    stats = sb.tile([128, 8], F32, tag=f"st{c}", bufs=1, name="stats")
    for m in range(FB):
        nc.vector.reduce_sum(stats[:, 2 * m:2 * m + 1], zt[:, m, :], axis=AX.X)
        scr = sb.tile([128, SH], F32, tag="scr", bufs=1, name="scr")
        nc.scalar.activation(scr[:], zt[:, m, :], AF.Square,
                             accum_out=stats[:, 2 * m + 1:2 * m + 2])
    ar_in = g["ar_in"][(c, l)]
    ar_out = g["ar_out"][(c, l)]
    nc.sync.dma_start(ar_in[:], stats[:])
    nc.gpsimd.collective_compute(
        "AllReduce", ALU.add, replica_groups=GROUPS,
        ins=[ar_in[:]], outs=[ar_out[:]])


def _emit_bn_apply(nc, g, c, l):
    """BN finalize from the AllReduced stats; fused BN+PReLU in place."""
    sb = g["sb"]
    li = LI[c][l]
    zt = g["zt"][c]
    gstats = sb.tile([128, 8], F32, tag="gstats", name="gstats")
    nc.sync.dma_start(gstats[:], g["ar_out"][(c, l)][:])
    mean = sb.tile([128, FB], F32, tag="mean", name="mean")
    var = sb.tile([128, FB], F32, tag="var", name="var")
    sN = sb.tile([128, FB], F32, tag="sN", name="sN")
    tN = sb.tile([128, FB], F32, tag="tN", name="tN")
    m2 = sb.tile([128, FB], F32, tag="m2", name="m2")
    nc.scalar.mul(mean[:], gstats[:, 0:8:2], 1.0 / N)
    nc.scalar.mul(var[:], gstats[:, 1:8:2], 1.0 / N)      # E[x^2]
    nc.vector.tensor_mul(m2[:], mean[:], mean[:])
    nc.vector.tensor_sub(var[:], var[:], m2[:])
    nc.scalar.activation(sN[:], var[:], AF.Sqrt, bias=g["epsb"][:])
    nc.vector.reciprocal(sN[:], sN[:])
    nc.vector.tensor_mul(sN[:], sN[:], g["g_sb"][:, li, :])
    nc.vector.tensor_mul(m2[:], mean[:], sN[:])
    nc.vector.tensor_sub(tN[:], g["bb_sb"][:, li, :], m2[:])
    for m in range(FB):
        nc.scalar.activation(
            zt[:, m, :], zt[:, m, :], AF.Prelu,
            bias=tN[:, m:m + 1], scale=sN[:, m:m + 1],
            alpha=g["al_sb"][:, 2 * li + 1:2 * li + 2])


def _emit_w_ag(nc, g, c, l):
    """p = (BN'd h) @ W_next (node-major out), fp8 quantize, AllGather."""
    sb, ps = g["sb"], g["ps"]
    zt = g["zt"][c]
    li_w = WNEXT[c][l]
    p_out = sb.tile([128, SB, F], F8, tag=f"po{c}", bufs=2, name="p_out")
    for wave in range(2):
        pps = [ps.tile([128, 512], F32, tag=f"ps{c}", bufs=4, name="pps")
               for _ in range(4)]
        for i in range(4):
            tb = wave * 4 + i
            for kb in range(FB):
                nc.tensor.matmul(
                    pps[i][:], zt[:, kb, tb * 128:(tb + 1) * 128],
                    g["w_sb"][:, li_w - 1, kb, :],
                    start=(kb == 0), stop=(kb == FB - 1))
        for i in range(4):
            tb = wave * 4 + i
            if l == 0:  # next layer is enc (normalized): scale by dsrc
                nc.vector.tensor_scalar_mul(
                    p_out[:, tb, :], pps[i][:], g["dsc"][c][:, tb:tb + 1])
            else:
                nc.vector.tensor_copy(p_out[:, tb, :], pps[i][:])
    agp_in = g["agp_in"][(c, l)]
    nc.sync.dma_start(agp_in[:], p_out[:])
    nc.gpsimd.collective_compute(
        "AllGather", ALU.bypass, replica_groups=GROUPS,
        ins=[agp_in[:]], outs=[g["agp_out"][(c, l)][:]])
    if (c, l) in g["dbg"]:
        nc.sync.dma_start(g["dbg"][(c, l)][:], p_out[:])


def _emit_tail1(nc, g):
    """Chain1 tail: loss1 partial = sum over shard of (1 - cos(h1, attr))^3."""
    sb, ps = g["sb"], g["ps"]
    zt = g["zt"][1]
    attrTn = g["attrTn"]
    dotps = [ps.tile([1, 512], F32, tag="ps1", bufs=4, name="dotps")
             for _ in range(2)]
    n1ps = [ps.tile([1, 512], F32, tag="ps1", bufs=4, name="n1ps")
            for _ in range(2)]
    for kb in range(FB):
        prod = sb.tile([128, SH], BF16, tag="prod", bufs=2, name="prod")
        nc.vector.tensor_tensor(prod[:], zt[:, kb, :], attrTn[:, kb, :],
                                ALU.mult)
        sq = sb.tile([128, SH], BF16, tag="sqh", bufs=2, name="sq")
        nc.gpsimd.tensor_tensor(sq[:], zt[:, kb, :], zt[:, kb, :], ALU.mult)
        for h in range(2):
            nc.tensor.matmul(dotps[h][:], g["ones128"][:],
                             prod[:, h * 512:(h + 1) * 512],
                             start=(kb == 0), stop=(kb == FB - 1))
            nc.tensor.matmul(n1ps[h][:], g["ones128"][:],
                             sq[:, h * 512:(h + 1) * 512],
                             start=(kb == 0), stop=(kb == FB - 1))
    dot_sb = sb.tile([1, SH], F32, tag="row", bufs=3, name="dot_sb")
    n1_sb = sb.tile([1, SH], F32, tag="row", bufs=3, name="n1_sb")
    for h in range(2):
        nc.vector.tensor_copy(dot_sb[:, h * 512:(h + 1) * 512], dotps[h][:])
        nc.vector.tensor_copy(n1_sb[:, h * 512:(h + 1) * 512], n1ps[h][:])
    nc.scalar.activation(n1_sb[:], n1_sb[:], AF.Sqrt)
    nc.vector.tensor_scalar_max(n1_sb[:], n1_sb[:], 1e-12)
    nc.vector.reciprocal(n1_sb[:], n1_sb[:])
    nc.vector.tensor_mul(dot_sb[:], dot_sb[:], n1_sb[:])        # cos
    u = sb.tile([1, SH], F32, tag="row", bufs=3, name="u")
    nc.scalar.activation(u[:], dot_sb[:], AF.Copy, scale=-1.0, bias=1.0)
    u2 = sb.tile([1, SH], F32, tag="row", bufs=3, name="u2")
    nc.vector.tensor_mul(u2[:], u[:], u[:])
    nc.vector.tensor_mul(u2[:], u2[:], u[:])                    # u^3
    l1p = sb.tile([1, 1], F32, tag="l1p", name="l1p")
    nc.vector.reduce_sum(l1p[:], u2[:], axis=AX.X)
    g["l1p"] = l1p


def _emit_tail2(nc, g):
    """Chain2 tail: l2-normalize h2 (via PE transposes), fp8, AllGather."""
    sb, ps = g["sb"], g["ps"]
    zt = g["zt"][2]
    h2nm = sb.tile([128, SB, F], BF16, tag="h2nm", bufs=1, name="h2nm")
    nrm2 = sb.tile([128, SB], F32, tag="nrm2", bufs=1, name="nrm2")
    for tb in range(SB):
        for m in range(FB):
            tp = ps.tile([128, 128], BF16, tag="ps2", bufs=4, name="tp")
            nc.tensor.transpose(tp[:], zt[:, m, tb * 128:(tb + 1) * 128],
                                g["ident"][:])
            nc.vector.tensor_copy(h2nm[:, tb, m * 128:(m + 1) * 128], tp[:])
        scr = sb.tile([128, 512], F32, tag="scrh", bufs=4, name="sqs")
        nc.scalar.activation(scr[:], h2nm[:, tb, :], AF.Square,
                             accum_out=nrm2[:, tb:tb + 1])
    nc.scalar.activation(nrm2[:], nrm2[:], AF.Sqrt)
    nc.vector.tensor_scalar_max(nrm2[:], nrm2[:], 1e-12)
    nc.vector.reciprocal(nrm2[:], nrm2[:])
    h2q = sb.tile([128, SB, F], F8, tag="h2q", bufs=1, name="h2q")
    for tb in range(SB):
        nc.vector.tensor_scalar_mul(h2q[:, tb, :], h2nm[:, tb, :],
                                    nrm2[:, tb:tb + 1])
    nc.sync.dma_start(g["ag2_in"][:], h2q[:])
    nc.gpsimd.collective_compute(
        "AllGather", ALU.bypass, replica_groups=GROUPS,
        ins=[g["ag2_in"][:]], outs=[g["ag2_out"][:]])
    g["h2q"] = h2q
    # local partial of G = H^T H (k over the local 8 node blocks), then
    # AllReduce it while the MH matmul runs
    gq = [ps.tile([128, 512], F32, tag="ps1", bufs=4, name="gq")
          for _ in range(FB)]
    for t in range(SB // 2):
        for mb in range(FB):
            nc.tensor.matmul(
                gq[mb][:], h2q[:, 2 * t:2 * t + 2, mb * 128:(mb + 1) * 128],
                h2q[:, 2 * t:2 * t + 2, :],
                start=(t == 0), stop=(t == SB // 2 - 1), perf_mode=DR)
    for mb in range(FB):
        gsc = sb.tile([128, 512], F32, tag="scrh", bufs=4, name="gsc")
        nc.vector.tensor_copy(gsc[:], gq[mb][:])
        nc.sync.dma_start(g["arg_in"][:, mb * 512:(mb + 1) * 512], gsc[:])
    nc.gpsimd.collective_compute(
        "AllReduce", ALU.add, replica_groups=GROUPS,
        ins=[g["arg_in"][:]], outs=[g["arg_out"][:]])
    if "h2q" in g["dbg"]:
        nc.sync.dma_start(g["dbg"]["h2q"][:], h2q[:])


def _emit_tail_mh_g(nc, g):
    """MH = M @ H (fp8 DR) with fused (MH*H) accumulation; partials."""
    sb, ps = g["sb"], g["ps"]
    # prefetch the first M^T chunks before the H reload
    mtcs = {}
    for t in range(3):
        mtc = sb.tile([128, 2, SH], F8, tag="mtc", bufs=3, name="mtc")
        nc.sync.dma_start(mtc[:], g["mt_dram"][:, 2 * t:2 * t + 2, :])
        mtcs[t] = mtc
    # gathered H in two halves (reusing the idle pf slots) so the MH matmul
    # starts after the first 2 MB instead of the full 4 MB
    hfA = sb.tile([128, NB // 2, F], F8, tag="pf2", bufs=1, name="hfA")
    hfB = sb.tile([128, NB // 2, F], F8, tag="pf1", bufs=1, name="hfB")
    for cc in range(NCORES):
        dst = hfA if cc < 4 else hfB
        nc.sync.dma_start(dst[:, 8 * (cc % 4):8 * (cc % 4) + 8, :],
                          g["ag2_out"][cc * 128:(cc + 1) * 128, :, :])
    mhps = [ps.tile([128, 512], F32, tag="ps1", bufs=4, name="mhps")
            for _ in range(4)]
    mhps += [ps.tile([128, 512], F32, tag="ps2", bufs=4, name="mhps2")
             for _ in range(4)]
    for t in range(32):
        if t in mtcs:
            mtc = mtcs[t]
        else:
            mtc = sb.tile([128, 2, SH], F8, tag="mtc", bufs=3, name="mtc")
            nc.sync.dma_start(mtc[:], g["mt_dram"][:, 2 * t:2 * t + 2, :])
        hf = hfA if t < 16 else hfB
        kk = 2 * t - (0 if t < 16 else 32)
        for ib in range(8):
            nc.tensor.matmul(
                mhps[ib][:], mtc[:, :, ib * 128:(ib + 1) * 128],
                hf[:, kk:kk + 2, :],
                start=(t == 0), stop=(t == 31), perf_mode=DR)
    xacc = sb.tile([128, 8], F32, tag="xacc", bufs=1, name="xacc")
    h2q = g["h2q"]
    for ib in range(8):
        scr = sb.tile([128, 512], F32, tag="scrh", bufs=4, name="xscr")
        nc.vector.scalar_tensor_tensor(
            scr[:], mhps[ib][:], 1.0, h2q[:, ib, :], op0=ALU.mult,
            op1=ALU.mult, accum_out=xacc[:, ib:ib + 1])
    # gsq from the AllReduced G partials (reuses the h2nm slot)
    gsb = sb.tile([128, FB, 512], F32, tag="h2nm", bufs=1, name="gsb")
    nc.sync.dma_start(gsb[:], g["arg_out"].rearrange("p (m f) -> p m f", m=FB))
    gacc = sb.tile([128, FB], F32, tag="gacc", bufs=1, name="gacc")
    for mb in range(FB):
        scr = sb.tile([128, 512], F32, tag="scrh", bufs=4, name="gscr")
        nc.scalar.activation(scr[:], gsb[:, mb, :], AF.Square,
                             accum_out=gacc[:, mb:mb + 1])
    g["xacc"] = xacc
    g["gacc"] = gacc


def _emit_partials(nc, g):
    sb, ps = g["sb"], g["ps"]
    # combine partials: [l1p_sum, cross_sum, gsq_sum]
    pl = sb.tile([128, 3], F32, tag="pl", name="pl")
    nc.vector.memset(pl[:], 0.0)
    if "l1p" in g:
        nc.vector.tensor_copy(pl[0:1, 0:1], g["l1p"][:])
    if "xacc" in g:
        nc.vector.reduce_sum(pl[:, 1:2], g["xacc"][:], axis=AX.X)
        nc.vector.reduce_sum(pl[:, 2:3], g["gacc"][:], axis=AX.X)
    pp = ps.tile([3, 1], F32, tag="ps1", bufs=4, name="pp")
    nc.tensor.matmul(pp[:], pl[:], g["onesf"][:], start=True, stop=True)
    out_sb = sb.tile([3, 1], F32, tag="out_sb", name="out_sb")
    nc.scalar.copy(out_sb[:], pp[:])
    nc.sync.dma_start(g["partials"][:], out_sb[:])


STOP_LV = {"l0h": 0, "l0s": 0.3, "l0w": 0.6, "l0": 1, "layers": 2, "tail1": 3,
           "tail2": 4, "full": 5}


def build_nc():
    nc = bacc.Bacc("TRN2", target_bir_lowering=False, debug=False,
                   num_devices=NCORES)

    ins = {}

    def di(name, shape, dt):
        ins[name] = nc.dram_tensor(name, shape, dt, kind="ExternalInput")
        return ins[name]

    p1_0 = di("p1_0", [128, NB, F], F8)
    p2_0 = di("p2_0", [128, NB, F], F8)
    a1 = di("a1", [2, 128, NB, 512], F8)
    a2 = di("a2", [2, 128, NB, 512], F8)
    mt = di("mt", [128, NB, SH], F8)
    w_all = di("w_all", [6, 128, FB, F], BF16)
    b_all = di("b_all", [6, F], F32)
    g_all = di("g_all", [6, F], F32)
    bb_all = di("bb_all", [6, F], F32)
    al_all = di("al_all", [1, 12], F32)
    ddb1 = di("ddb1", [128, SH], F32)
    ddb2 = di("ddb2", [128, SH], F32)
    dsc1 = di("dsc1", [128, SB], F32)
    dsc2 = di("dsc2", [128, SB], F32)
    attrTn = di("attrTn", [128, FB, SH], BF16)

    partials = nc.dram_tensor("partials", [3, 1], F32, kind="ExternalOutput")

    ar_in, ar_out, agp_in, agp_out = {}, {}, {}, {}
    for c in (1, 2):
        for l in range(4):
            ar_in[(c, l)] = nc.dram_tensor(f"ar_in_{c}_{l}", [128, 8], F32)
            ar_out[(c, l)] = nc.dram_tensor(f"ar_out_{c}_{l}", [128, 8], F32,
                                            addr_space="Shared")
            if l < 3:
                agp_in[(c, l)] = nc.dram_tensor(f"agp_in_{c}_{l}",
                                                [128, SB, F], F8)
                agp_out[(c, l)] = nc.dram_tensor(f"agp_out_{c}_{l}",
                                                 [NCORES * 128, SB, F], F8,
                                                 addr_space="Shared")
    ag2_in = nc.dram_tensor("ag2_in", [128, SB, F], F8)
    ag2_out = nc.dram_tensor("ag2_out", [NCORES * 128, SB, F], F8,
                             addr_space="Shared")
    warm_in = nc.dram_tensor("warm_in", [128, 8], F32)
    warm_out = nc.dram_tensor("warm_out", [128, 8], F32, addr_space="Shared")
    arg_in = nc.dram_tensor("arg_in", [128, FB * 512], F32)
    arg_out = nc.dram_tensor("arg_out", [128, FB * 512], F32,
                             addr_space="Shared")

    dbg = {}
    if os.environ.get("BASSK_DEBUG"):
        for c in (1, 2):
            for l in range(3):
                dbg[(c, l)] = nc.dram_tensor(f"dbg_p_{c}_{l}", [128, SB, F],
                                             F8, kind="ExternalOutput")
        dbg["h2q"] = nc.dram_tensor("dbg_h2q", [128, SB, F], F8,
                                    kind="ExternalOutput")

    with tile.TileContext(nc) as tc:
        with (
            tc.tile_pool(name="sb", bufs=2) as sb,
            tc.tile_pool(name="ps", bufs=4, space="PSUM") as ps,
        ):
            g = {
                "sb": sb, "ps": ps, "dbg": dbg, "partials": partials,
                "a_dram": {1: a1, 2: a2}, "mt_dram": mt,
                "p0": {1: p1_0, 2: p2_0},
                "ar_in": ar_in, "ar_out": ar_out,
                "agp_in": agp_in, "agp_out": agp_out,
                "ag2_in": ag2_in, "ag2_out": ag2_out,
                "pf": {}, "zt": {},
                "warm_in": warm_in, "warm_out": warm_out,
                "arg_in": arg_in, "arg_out": arg_out,
            }
            # ---- constants / params ----
            ident = sb.tile([128, 128], BF16, tag="ident", bufs=1, name="ident")
            make_identity(nc, ident[:])
            g["ident"] = ident
            for nm, src in (("b_sb", b_all), ("g_sb", g_all), ("bb_sb", bb_all)):
                t = sb.tile([128, 6, FB], F32, tag=nm, bufs=1, name=nm)
                nc.sync.dma_start(t[:], src.rearrange("l (m p) -> p l m", p=128))
                g[nm] = t
            al1 = sb.tile([1, 12], F32, tag="al1", bufs=1, name="al1")
            nc.sync.dma_start(al1[:], al_all[:])
            al_sb = sb.tile([128, 12], F32, tag="al_sb", bufs=1, name="al_sb")
            nc.gpsimd.partition_broadcast(al_sb[:], al1[:])
            g["al_sb"] = al_sb
            epsb = sb.tile([128, 1], F32, tag="epsb", bufs=1, name="epsb")
            nc.vector.memset(epsb[:], 1e-5)
            g["epsb"] = epsb
            ones128 = sb.tile([128, 1], BF16, tag="ones128", bufs=1, name="ones128")
            nc.vector.memset(ones128[:], 1.0)
            g["ones128"] = ones128
            onesf = sb.tile([128, 1], F32, tag="onesf", bufs=1, name="onesf")
            nc.vector.memset(onesf[:], 1.0)
            g["onesf"] = onesf
            # warm up the collective path during the startup loads
            if os.environ.get("BASSK_WARM"):
                # no measured benefit: early-collective latency jitter persists
                wsb = sb.tile([128, 8], F32, tag="wsb", bufs=1, name="wsb")
                nc.vector.memset(wsb[:], 0.0)
                nc.sync.dma_start(g["warm_in"][:], wsb[:])
                nc.gpsimd.collective_compute(
                    "AllReduce", ALU.add, replica_groups=GROUPS,
                    ins=[g["warm_in"][:]], outs=[g["warm_out"][:]])

            # ---- staggered 2-chain layer pipeline ----
            EM = STOP_LV[os.environ.get("BASSK_STOP", "full")]
            _emit_pf_load(nc, g, 1, 0)
            # ddb1 is needed by the first eviction; the rest after the A DMAs
            ddbt = {}
            t1 = sb.tile([128, SH], F32, tag="ddb1", bufs=1, name="ddb")
            nc.sync.dma_start(t1[:], ddb1[:])
            ddbt[1] = t1
            g["ddb"] = ddbt
            _emit_A_half(nc, g, 1, 0, 0)
            t2 = sb.tile([128, SH], F32, tag="ddb2", bufs=1, name="ddb")
            nc.sync.dma_start(t2[:], ddb2[:])
            ddbt[2] = t2
            dsct = {}
            for c, src in ((1, dsc1), (2, dsc2)):
                t = sb.tile([128, SB], F32, tag=f"dsc{c}", bufs=1, name="dsc")
                nc.sync.dma_start(t[:], src[:])
                dsct[c] = t
            g["dsc"] = dsct
            w_sb = sb.tile([128, 5, FB, F], BF16, tag="w_sb", bufs=1, name="w_sb")
            for li in (1, 2, 3, 4, 5):  # row 0 (enc0) applied on host
                nc.sync.dma_start(w_sb[:, li - 1], w_all[li])
            g["w_sb"] = w_sb
            at = sb.tile([128, FB, SH], BF16, tag="attrTn", bufs=1, name="attrTn")
            nc.sync.dma_start(at[:], attrTn[:])
            g["attrTn"] = at
            if EM >= 0.3:
                _emit_A_half(nc, g, 1, 0, 1)
                _emit_stats_ar(nc, g, 1, 0)
            if EM >= 0.6:
                _emit_pf_load(nc, g, 2, 0)
                _emit_A_half(nc, g, 2, 0, 0)
                _emit_bn_apply(nc, g, 1, 0)
                _emit_w_ag(nc, g, 1, 0)
            if EM >= 1:
                _emit_A_half(nc, g, 2, 0, 1)
                _emit_stats_ar(nc, g, 2, 0)
            if EM >= 2:
                for l in range(1, 4):
                    _emit_pf_load(nc, g, 1, l)
                    _emit_A_half(nc, g, 1, l, 0)
                    _emit_bn_apply(nc, g, 2, l - 1)
                    _emit_w_ag(nc, g, 2, l - 1)
                    _emit_A_half(nc, g, 1, l, 1)
                    _emit_stats_ar(nc, g, 1, l)
                    _emit_pf_load(nc, g, 2, l)
                    _emit_A_half(nc, g, 2, l, 0)
                    if l < 3:
                        _emit_bn_apply(nc, g, 1, l)
                        _emit_w_ag(nc, g, 1, l)
                        _emit_A_half(nc, g, 2, l, 1)
                        _emit_stats_ar(nc, g, 2, l)
                    else:
                        _emit_A_half(nc, g, 2, l, 1)
                        _emit_stats_ar(nc, g, 2, l)
                        _emit_bn_apply(nc, g, 1, l)
                        if EM >= 3:
                            _emit_tail1(nc, g)
            if EM >= 4:
                _emit_bn_apply(nc, g, 2, 3)
                _emit_tail2(nc, g)
            if EM >= 5:
                _emit_tail_mh_g(nc, g)
            _emit_partials(nc, g)

    nc.compile()
    return nc


_NC_CACHE = None


def _get_nc():
    global _NC_CACHE
    if _NC_CACHE is None:
        _NC_CACHE = build_nc()
    return _NC_CACHE


def _dinv(idx):
    deg = np.bincount(idx, minlength=N).astype(np.float32)
    return 1.0 / np.sqrt(np.clip(deg, 1.0, None))


def _adj_t(src, dst):
    """A^T[s, d] = multiplicity of edge s->d, float32 [N, N]."""
    flat = src.astype(np.int64) * N + dst.astype(np.int64)
    return np.bincount(flat, minlength=N * N).astype(np.float32).reshape(N, N)


def _swz_nodes(x, width):
    """[8192, width] -> [128, 64, width] with node = t*128 + p."""
    return np.ascontiguousarray(
        x.reshape(NB, 128, width).transpose(1, 0, 2))


def _swz_a(x):
    """[8192, 1024] -> [2, 128, 64, 512]: per dest-half, contiguous chunks."""
    sw = x.reshape(NB, 128, 2, 512).transpose(2, 0, 1, 3)
    return np.ascontiguousarray(sw.transpose(0, 2, 1, 3))


def host_prep(inputs):
    f8 = ml_dtypes.float8_e4m3
    bf16 = ml_dtypes.bfloat16
    attr = np.asarray(inputs["attr"], np.float32)
    matrix = np.asarray(inputs["matrix"], np.float32)
    mask1 = np.asarray(inputs["enc_mask_token1"], np.float32)
    src = np.asarray(inputs["src"]); dst = np.asarray(inputs["dst"])
    src2 = np.asarray(inputs["src2"]); dst2 = np.asarray(inputs["dst2"])
    tok = np.asarray(inputs["token_nodes"])
    noi = np.asarray(inputs["noise_nodes"])
    nsrc = np.asarray(inputs["noise_src"])

    x = attr.copy()
    x[tok] = 0.0
    x[noi] = attr[nsrc]
    np.add.at(x, tok, mask1[0])

    d1s, d1d = _dinv(src), _dinv(dst)
    d2s, d2d = _dinv(src2), _dinv(dst2)

    a1t = _adj_t(src, dst)    # A^T[s, d]
    a2t = _adj_t(src2, dst2)

    W0 = np.asarray(inputs["enc_W"][0], np.float32)
    p1_0 = _swz_nodes((d1s[:, None] * (x @ W0)).astype(f8), F)
    p2_0 = _swz_nodes((d2s[:, None] * (attr @ W0)).astype(f8), F)

    # w rows: enc0 enc1 d10 d11 d20 d21; device layout [6, 128, 4, 512]
    w_list = [np.asarray(inputs["enc_W"][0]), np.asarray(inputs["enc_W"][1]),
              np.asarray(inputs["dec1_W"][0]), np.asarray(inputs["dec1_W"][1]),
              np.asarray(inputs["dec2_W"][0]), np.asarray(inputs["dec2_W"][1])]
    w_all = np.ascontiguousarray(np.stack(
        [w.reshape(FB, 128, F).transpose(1, 0, 2) for w in w_list]
    ).astype(bf16))

    def stack6(key):
        return np.stack([
            np.asarray(inputs[f"enc_{key}"][0]), np.asarray(inputs[f"enc_{key}"][1]),
            np.asarray(inputs[f"dec1_{key}"][0]), np.asarray(inputs[f"dec1_{key}"][1]),
            np.asarray(inputs[f"dec2_{key}"][0]), np.asarray(inputs[f"dec2_{key}"][1]),
        ]).astype(np.float32)

    b_all, g_all, bb_all = stack6("b"), stack6("g"), stack6("bb")
    al = np.zeros((1, 12), np.float32)
    for i, (sa, so) in enumerate((("enc", 0), ("enc", 1), ("dec1", 0),
                                  ("dec1", 1), ("dec2", 0), ("dec2", 1))):
        al[0, 2 * i] = np.asarray(inputs[f"{sa}_ain"])[so]
        al[0, 2 * i + 1] = np.asarray(inputs[f"{sa}_aout"])[so]

    an = attr / np.maximum(np.linalg.norm(attr, axis=-1, keepdims=True), 1e-12)
    sumM2 = float(np.sum(matrix.astype(np.float64) ** 2))

    a1q = a1t.astype(f8)
    a2q = a2t.astype(f8)

    in_maps = []
    for c in range(NCORES):
        sl = slice(c * SH, (c + 1) * SH)
        # attrTn: feature-major [128, 4, 1024] for this shard
        at_sh = np.ascontiguousarray(
            an[sl].T.reshape(FB, 128, SH).transpose(1, 0, 2)).astype(bf16)
        in_maps.append({
            "p1_0": p1_0, "p2_0": p2_0,
            "a1": _swz_a(a1q[:, sl]),
            "a2": _swz_a(a2q[:, sl]),
            "mt": _swz_nodes(
                np.ascontiguousarray(matrix[sl].T).astype(f8), SH),
            "w_all": w_all, "b_all": b_all, "g_all": g_all, "bb_all": bb_all,
            "al_all": al,
            "ddb1": np.ascontiguousarray(
                np.broadcast_to(d1d[sl], (128, SH))).astype(np.float32),
            "ddb2": np.ascontiguousarray(
                np.broadcast_to(d2d[sl], (128, SH))).astype(np.float32),
            "dsc1": np.ascontiguousarray(d1s[sl].reshape(SB, 128).T),
            "dsc2": np.ascontiguousarray(d2s[sl].reshape(SB, 128).T),
            "attrTn": at_sh,
        })
    return in_maps, sumM2


def combine(results, sumM2):
    l1 = sum(float(r["partials"][0, 0]) for r in results)
    cross = sum(float(r["partials"][1, 0]) for r in results)
    gsq = np.mean([float(r["partials"][2, 0]) for r in results])
    loss1 = l1 / N
    loss2 = (sumM2 - 2.0 * cross + gsq) / (float(N) * N)
    return np.asarray(0.5 * loss1 + 0.5 * loss2, dtype=np.float32)


def run(inputs, trace=False, trace_kwargs=None):
    nc = _get_nc()
    in_maps, sumM2 = host_prep(inputs)
    res = run_bass_kernel_spmd(nc, in_maps, core_ids=list(range(NCORES)),
                               trace=trace, **(trace_kwargs or {}))
    return combine(res.results, sumM2), res


def kernel(**inputs) -> np.ndarray:
    out, _ = run(inputs, trace=False)
    return out


# revision 33
# speedup vs baseline: 1.0308x; 1.0285x over previous
"""Trainium2 Bass kernel for the GNN message-passing autoencoder problem.

Strategy (8 NeuronCores, SPMD), v2 (fp8):
  - Nodes sharded 1024/core. Message passing is a dense matmul against the
    PLAIN adjacency transpose shard A^T[:, shard] in fp8 e4m3 (counts are
    exact in fp8) using DoubleRow perf mode. GraphConv 'both' norms are
    folded into per-node scalings: D_src^-1/2 is applied to the (h @ W)
    activations (exact per-partition scale), D_dst^-1/2 multiplies the
    aggregation PSUM before bias+PReLU.
  - The per-layer linear W is applied BEFORE the AllGather (z = A (h W) ==
    (A h) W): lhsT = feature-major BN'd h, rhs = W, giving node-major
    activations p directly - no PE transposes in the layer loop. p is
    quantized to fp8 and AllGathered (4 MB full graph).
  - Layer epilogue: bias+PReLU fused in the PSUM eviction (scalar engine),
    BN stats partials AllReduced (4 KB), BN+PReLU fused in one activation.
  - The two chains are interleaved with a half-layer stagger so ARs/AGs hide
    under the other chain's matmuls.
  - Tail: loss2*N^2 = sum(M^2) - 2*tr(H^T M H) + ||H^T H||_F^2 with
    H = l2-normalized h2 in fp8. sum(M^2) on host; tr term via an fp8
    DoubleRow matmul (M^T shard stationary, gathered H moving) with a fused
    multiply-accumulate eviction; G = H^T H computed redundantly per core.
    loss1 (cosine^3) is computed per-shard in feature-major layout using
    ones-vector matmuls for the partition reductions.
"""

import os
import sys

for _p in ("/opt/trn_rl_repo", "/opt/pypackages"):
    if _p not in sys.path:
        sys.path.append(_p)

import numpy as np
import ml_dtypes

import concourse.bass as bass
import concourse.mybir as mybir
import concourse.tile as tile
from concourse import bacc
from concourse.bass_utils import run_bass_kernel_spmd
from concourse.masks import make_identity

F8 = mybir.dt.float8e4
BF16 = mybir.dt.bfloat16
F32 = mybir.dt.float32
AF = mybir.ActivationFunctionType
ALU = mybir.AluOpType
AX = mybir.AxisListType
DR = mybir.MatmulPerfMode.DoubleRow

N = 8192
F = 512
NCORES = 8
SH = N // NCORES          # 1024 nodes per core shard
NB = N // 128             # 64 node k-subtiles
SB = SH // 128            # 8 node blocks per shard
FB = F // 128             # 4 feature blocks
GROUPS = [list(range(NCORES))]

# layer-instance parameter rows: enc0 enc1 dec1_0 dec1_1 dec2_0 dec2_1
LI = {1: [0, 1, 2, 3], 2: [0, 1, 4, 5]}
# W row applied at the END of layer l (producing p for layer l+1)
WNEXT = {1: [1, 2, 3, None], 2: [1, 4, 5, None]}


def _emit_pf_load(nc, g, c, l):
    """Load the full-graph node-major fp8 activations for layer l."""
    sb = g["sb"]
    pf = sb.tile([128, NB, F], F8, tag=f"pf{c}", bufs=1, name="pf")
    if l == 0:
        src = g["p0"][c]
        for q in range(8):
            nc.sync.dma_start(pf[:, 8 * q:8 * q + 8, :],
                              src[:, 8 * q:8 * q + 8, :])
    else:
        # issue on gpsimd: these wait on the AllGather, and on the in-order
        # sync queue they head-of-line block the independent A-chunk stream
        src = g["agp_out"][(c, l - 1)]
        for cc in range(NCORES):
            nc.gpsimd.dma_start(pf[:, 8 * cc:8 * cc + 8, :],
                                src[cc * 128:(cc + 1) * 128, :, :])
    g["pf"][c] = pf


def _emit_A_half(nc, g, c, l, half):
    """A-aggregation matmuls for one 512-dest half; evict with bias+PReLU."""
    sb, ps = g["sb"], g["ps"]
    li = LI[c][l]
    pf = g["pf"][c]
    a_dram = g["a_dram"][c]
    if half == 0:
        zt = sb.tile([128, FB, SH], BF16, tag=f"zt{c}", bufs=1, name="zt")
        g["zt"][c] = zt
    else:
        zt = g["zt"][c]
    zps = [ps.tile([128, 512], F32, tag=f"ps{c}", bufs=4, name="zps")
           for _ in range(FB)]
    for th in range(16):
        art = sb.tile([128, 4, 512], F8, tag=f"a{c}", bufs=3, name="art")
        nc.sync.dma_start(art[:], a_dram[half, :, 4 * th:4 * th + 4, :])
        for j in range(2):
            kp = 2 * th + j
            kk = 4 * th + 2 * j
            for m in range(FB):
                nc.tensor.matmul(
                    zps[m][:],
                    pf[:, kk:kk + 2, m * 128:(m + 1) * 128],
                    art[:, 2 * j:2 * j + 2, :],
                    start=(kp == 0), stop=(kp == 31), perf_mode=DR)
    for m in range(FB):
        dst = zt[:, m, half * 512:(half + 1) * 512]
        bias = g["b_sb"][:, li, m:m + 1]
        alpha = g["al_sb"][:, 2 * li:2 * li + 1]
        if l < 2:  # enc layer: multiply by ddst before bias+prelu
            zsc = sb.tile([128, 512], F32, tag="scrh", bufs=4, name="zsc")
            nc.vector.tensor_tensor(
                zsc[:], zps[m][:],
                g["ddb"][c][:, half * 512:(half + 1) * 512], ALU.mult)
            nc.scalar.activation(dst, zsc[:], AF.Prelu, bias=bias, scale=1.0,
                                 alpha=alpha)
        else:
            nc.scalar.activation(dst, zps[m][:], AF.Prelu, bias=bias,
                                 scale=1.0, alpha=alpha)


def _emit_stats_ar(nc, g, c, l):
    """Per-core BN stats (sum, sumsq per feature) and the AllReduce."""
    sb = g["sb"]
    zt = g["zt"][c]
    stats = sb.tile([128, 8], F32, tag=f"st{c}", bufs=1, name="stats")
    for m in range(FB):
        nc.vector.reduce_sum(stats[:, 2 * m:2 * m + 1], zt[:, m, :], axis=AX.X)
        scr = sb.tile([128, SH], F32, tag="scr", bufs=1, name="scr")
        nc.scalar.activation(scr[:], zt[:, m, :], AF.Square,
                             accum_out=stats[:, 2 * m + 1:2 * m + 2])
    ar_in = g["ar_in"][(c, l)]
    ar_out = g["ar_out"][(c, l)]
    nc.sync.dma_start(ar_in[:], stats[:])
    nc.gpsimd.collective_compute(
        "AllReduce", ALU.add, replica_groups=GROUPS,
        ins=[ar_in[:]], outs=[ar_out[:]])


def _emit_bn_apply(nc, g, c, l):
    """BN finalize from the AllReduced stats; fused BN+PReLU in place."""
    sb = g["sb"]
    li = LI[c][l]
    zt = g["zt"][c]
    gstats = sb.tile([128, 8], F32, tag="gstats", name="gstats")
    nc.gpsimd.dma_start(gstats[:], g["ar_out"][(c, l)][:])
    mean = sb.tile([128, FB], F32, tag="mean", name="mean")
    var = sb.tile([128, FB], F32, tag="var", name="var")
    sN = sb.tile([128, FB], F32, tag="sN", name="sN")
    tN = sb.tile([128, FB], F32, tag="tN", name="tN")
    m2 = sb.tile([128, FB], F32, tag="m2", name="m2")
    nc.scalar.mul(mean[:], gstats[:, 0:8:2], 1.0 / N)
    nc.scalar.mul(var[:], gstats[:, 1:8:2], 1.0 / N)      # E[x^2]
    nc.vector.tensor_mul(m2[:], mean[:], mean[:])
    nc.vector.tensor_sub(var[:], var[:], m2[:])
    nc.scalar.activation(sN[:], var[:], AF.Sqrt, bias=g["epsb"][:])
    nc.vector.reciprocal(sN[:], sN[:])
    nc.vector.tensor_mul(sN[:], sN[:], g["g_sb"][:, li, :])
    nc.vector.tensor_mul(m2[:], mean[:], sN[:])
    nc.vector.tensor_sub(tN[:], g["bb_sb"][:, li, :], m2[:])
    for m in range(FB):
        nc.scalar.activation(
            zt[:, m, :], zt[:, m, :], AF.Prelu,
            bias=tN[:, m:m + 1], scale=sN[:, m:m + 1],
            alpha=g["al_sb"][:, 2 * li + 1:2 * li + 2])


def _emit_w_ag(nc, g, c, l):
    """p = (BN'd h) @ W_next (node-major out), fp8 quantize, AllGather."""
    sb, ps = g["sb"], g["ps"]
    zt = g["zt"][c]
    li_w = WNEXT[c][l]
    p_out = sb.tile([128, SB, F], F8, tag=f"po{c}", bufs=2, name="p_out")
    for wave in range(2):
        pps = [ps.tile([128, 512], F32, tag=f"ps{c}", bufs=4, name="pps")
               for _ in range(4)]
        for i in range(4):
            tb = wave * 4 + i
            for kb in range(FB):
                nc.tensor.matmul(
                    pps[i][:], zt[:, kb, tb * 128:(tb + 1) * 128],
                    g["w_sb"][:, li_w - 1, kb, :],
                    start=(kb == 0), stop=(kb == FB - 1))
        for i in range(4):
            tb = wave * 4 + i
            if l == 0:  # next layer is enc (normalized): scale by dsrc
                nc.vector.tensor_scalar_mul(
                    p_out[:, tb, :], pps[i][:], g["dsc"][c][:, tb:tb + 1])
            else:
                nc.vector.tensor_copy(p_out[:, tb, :], pps[i][:])
    agp_in = g["agp_in"][(c, l)]
    nc.sync.dma_start(agp_in[:], p_out[:])
    nc.gpsimd.collective_compute(
        "AllGather", ALU.bypass, replica_groups=GROUPS,
        ins=[agp_in[:]], outs=[g["agp_out"][(c, l)][:]])
    if (c, l) in g["dbg"]:
        nc.sync.dma_start(g["dbg"][(c, l)][:], p_out[:])


def _emit_tail1(nc, g):
    """Chain1 tail: loss1 partial = sum over shard of (1 - cos(h1, attr))^3."""
    sb, ps = g["sb"], g["ps"]
    zt = g["zt"][1]
    attrTn = g["attrTn"]
    dotps = [ps.tile([1, 512], F32, tag="ps1", bufs=4, name="dotps")
             for _ in range(2)]
    n1ps = [ps.tile([1, 512], F32, tag="ps1", bufs=4, name="n1ps")
            for _ in range(2)]
    for kb in range(FB):
        prod = sb.tile([128, SH], BF16, tag="prod", bufs=2, name="prod")
        nc.vector.tensor_tensor(prod[:], zt[:, kb, :], attrTn[:, kb, :],
                                ALU.mult)
        sq = sb.tile([128, SH], BF16, tag="sqh", bufs=2, name="sq")
        nc.gpsimd.tensor_tensor(sq[:], zt[:, kb, :], zt[:, kb, :], ALU.mult)
        for h in range(2):
            nc.tensor.matmul(dotps[h][:], g["ones128"][:],
                             prod[:, h * 512:(h + 1) * 512],
                             start=(kb == 0), stop=(kb == FB - 1))
            nc.tensor.matmul(n1ps[h][:], g["ones128"][:],
                             sq[:, h * 512:(h + 1) * 512],
                             start=(kb == 0), stop=(kb == FB - 1))
    dot_sb = sb.tile([1, SH], F32, tag="row", bufs=3, name="dot_sb")
    n1_sb = sb.tile([1, SH], F32, tag="row", bufs=3, name="n1_sb")
    for h in range(2):
        nc.vector.tensor_copy(dot_sb[:, h * 512:(h + 1) * 512], dotps[h][:])
        nc.vector.tensor_copy(n1_sb[:, h * 512:(h + 1) * 512], n1ps[h][:])
    nc.scalar.activation(n1_sb[:], n1_sb[:], AF.Sqrt)
    nc.vector.tensor_scalar_max(n1_sb[:], n1_sb[:], 1e-12)
    nc.vector.reciprocal(n1_sb[:], n1_sb[:])
    nc.vector.tensor_mul(dot_sb[:], dot_sb[:], n1_sb[:])        # cos
    u = sb.tile([1, SH], F32, tag="row", bufs=3, name="u")
    nc.scalar.activation(u[:], dot_sb[:], AF.Copy, scale=-1.0, bias=1.0)
    u2 = sb.tile([1, SH], F32, tag="row", bufs=3, name="u2")
    nc.vector.tensor_mul(u2[:], u[:], u[:])
    nc.vector.tensor_mul(u2[:], u2[:], u[:])                    # u^3
    l1p = sb.tile([1, 1], F32, tag="l1p", name="l1p")
    nc.vector.reduce_sum(l1p[:], u2[:], axis=AX.X)
    g["l1p"] = l1p


def _emit_tail2(nc, g):
    """Chain2 tail: l2-normalize h2 (via PE transposes), fp8, AllGather."""
    sb, ps = g["sb"], g["ps"]
    zt = g["zt"][2]
    h2nm = sb.tile([128, SB, F], BF16, tag="h2nm", bufs=1, name="h2nm")
    nrm2 = sb.tile([128, SB], F32, tag="nrm2", bufs=1, name="nrm2")
    for tb in range(SB):
        for m in range(FB):
            tp = ps.tile([128, 128], BF16, tag="ps2", bufs=4, name="tp")
            nc.tensor.transpose(tp[:], zt[:, m, tb * 128:(tb + 1) * 128],
                                g["ident"][:])
            nc.vector.tensor_copy(h2nm[:, tb, m * 128:(m + 1) * 128], tp[:])
        scr = sb.tile([128, 512], F32, tag="scrh", bufs=4, name="sqs")
        nc.scalar.activation(scr[:], h2nm[:, tb, :], AF.Square,
                             accum_out=nrm2[:, tb:tb + 1])
    nc.scalar.activation(nrm2[:], nrm2[:], AF.Sqrt)
    nc.vector.tensor_scalar_max(nrm2[:], nrm2[:], 1e-12)
    nc.vector.reciprocal(nrm2[:], nrm2[:])
    h2q = sb.tile([128, SB, F], F8, tag="h2q", bufs=1, name="h2q")
    for tb in range(SB):
        nc.vector.tensor_scalar_mul(h2q[:, tb, :], h2nm[:, tb, :],
                                    nrm2[:, tb:tb + 1])
    nc.sync.dma_start(g["ag2_in"][:], h2q[:])
    nc.gpsimd.collective_compute(
        "AllGather", ALU.bypass, replica_groups=GROUPS,
        ins=[g["ag2_in"][:]], outs=[g["ag2_out"][:]])
    g["h2q"] = h2q
    # local partial of G = H^T H (k over the local 8 node blocks), then
    # AllReduce it while the MH matmul runs
    gq = [ps.tile([128, 512], F32, tag="ps1", bufs=4, name="gq")
          for _ in range(FB)]
    for t in range(SB // 2):
        for mb in range(FB):
            nc.tensor.matmul(
                gq[mb][:], h2q[:, 2 * t:2 * t + 2, mb * 128:(mb + 1) * 128],
                h2q[:, 2 * t:2 * t + 2, :],
                start=(t == 0), stop=(t == SB // 2 - 1), perf_mode=DR)
    for mb in range(FB):
        gsc = sb.tile([128, 512], F32, tag="scrh", bufs=4, name="gsc")
        nc.vector.tensor_copy(gsc[:], gq[mb][:])
        nc.sync.dma_start(g["arg_in"][:, mb * 512:(mb + 1) * 512], gsc[:])
    nc.gpsimd.collective_compute(
        "AllReduce", ALU.add, replica_groups=GROUPS,
        ins=[g["arg_in"][:]], outs=[g["arg_out"][:]])
    if "h2q" in g["dbg"]:
        nc.sync.dma_start(g["dbg"]["h2q"][:], h2q[:])


def _emit_tail_mh_g(nc, g):
    """MH = M @ H (fp8 DR) with fused (MH*H) accumulation; partials."""
    sb, ps = g["sb"], g["ps"]
    # prefetch the first M^T chunks before the H reload
    mtcs = {}
    for t in range(2):
        mtc = sb.tile([128, 2, SH], F8, tag="mtc", bufs=3, name="mtc")
        nc.sync.dma_start(mtc[:], g["mt_dram"][:, 2 * t:2 * t + 2, :])
        mtcs[t] = mtc
    hf = sb.tile([128, NB, F], F8, tag="pf2", bufs=1, name="hf")
    for cc in range(NCORES):
        nc.gpsimd.dma_start(hf[:, 8 * cc:8 * cc + 8, :],
                            g["ag2_out"][cc * 128:(cc + 1) * 128, :, :])
    mhps = [ps.tile([128, 512], F32, tag="ps1", bufs=4, name="mhps")
            for _ in range(4)]
    mhps += [ps.tile([128, 512], F32, tag="ps2", bufs=4, name="mhps2")
             for _ in range(4)]
    for t in range(32):
        if t in mtcs:
            mtc = mtcs[t]
        else:
            mtc = sb.tile([128, 2, SH], F8, tag="mtc", bufs=3, name="mtc")
            nc.sync.dma_start(mtc[:], g["mt_dram"][:, 2 * t:2 * t + 2, :])
        for ib in range(8):
            nc.tensor.matmul(
                mhps[ib][:], mtc[:, :, ib * 128:(ib + 1) * 128],
                hf[:, 2 * t:2 * t + 2, :],
                start=(t == 0), stop=(t == 31), perf_mode=DR)
    xacc = sb.tile([128, 8], F32, tag="xacc", bufs=1, name="xacc")
    h2q = g["h2q"]
    for ib in range(8):
        scr = sb.tile([128, 512], F32, tag="scrh", bufs=4, name="xscr")
        nc.vector.scalar_tensor_tensor(
            scr[:], mhps[ib][:], 1.0, h2q[:, ib, :], op0=ALU.mult,
            op1=ALU.mult, accum_out=xacc[:, ib:ib + 1])
    # gsq from the AllReduced G partials (reuses the h2nm slot)
    gsb = sb.tile([128, FB, 512], F32, tag="h2nm", bufs=1, name="gsb")
    nc.sync.dma_start(gsb[:], g["arg_out"].rearrange("p (m f) -> p m f", m=FB))
    gacc = sb.tile([128, FB], F32, tag="gacc", bufs=1, name="gacc")
    for mb in range(FB):
        scr = sb.tile([128, 512], F32, tag="scrh", bufs=4, name="gscr")
        nc.scalar.activation(scr[:], gsb[:, mb, :], AF.Square,
                             accum_out=gacc[:, mb:mb + 1])
    g["xacc"] = xacc
    g["gacc"] = gacc


def _emit_partials(nc, g):
    sb, ps = g["sb"], g["ps"]
    # combine partials: [l1p_sum, cross_sum, gsq_sum]
    pl = sb.tile([128, 3], F32, tag="pl", name="pl")
    nc.vector.memset(pl[:], 0.0)
    if "l1p" in g:
        nc.vector.tensor_copy(pl[0:1, 0:1], g["l1p"][:])
    if "xacc" in g:
        nc.vector.reduce_sum(pl[:, 1:2], g["xacc"][:], axis=AX.X)
        nc.vector.reduce_sum(pl[:, 2:3], g["gacc"][:], axis=AX.X)
    pp = ps.tile([3, 1], F32, tag="ps1", bufs=4, name="pp")
    nc.tensor.matmul(pp[:], pl[:], g["onesf"][:], start=True, stop=True)
    out_sb = sb.tile([3, 1], F32, tag="out_sb", name="out_sb")
    nc.scalar.copy(out_sb[:], pp[:])
    nc.sync.dma_start(g["partials"][:], out_sb[:])


STOP_LV = {"l0h": 0, "l0s": 0.3, "l0w": 0.6, "l0": 1, "layers": 2, "tail1": 3,
           "tail2": 4, "full": 5}


def build_nc():
    nc = bacc.Bacc("TRN2", target_bir_lowering=False, debug=False,
                   num_devices=NCORES)

    ins = {}

    def di(name, shape, dt):
        ins[name] = nc.dram_tensor(name, shape, dt, kind="ExternalInput")
        return ins[name]

    p1_0 = di("p1_0", [128, NB, F], F8)
    p2_0 = di("p2_0", [128, NB, F], F8)
    a1 = di("a1", [2, 128, NB, 512], F8)
    a2 = di("a2", [2, 128, NB, 512], F8)
    mt = di("mt", [128, NB, SH], F8)
    w_all = di("w_all", [6, 128, FB, F], BF16)
    b_all = di("b_all", [6, F], F32)
    g_all = di("g_all", [6, F], F32)
    bb_all = di("bb_all", [6, F], F32)
    al_all = di("al_all", [1, 12], F32)
    ddb1 = di("ddb1", [128, SH], F32)
    ddb2 = di("ddb2", [128, SH], F32)
    dsc1 = di("dsc1", [128, SB], F32)
    dsc2 = di("dsc2", [128, SB], F32)
    attrTn = di("attrTn", [128, FB, SH], BF16)

    partials = nc.dram_tensor("partials", [3, 1], F32, kind="ExternalOutput")

    ar_in, ar_out, agp_in, agp_out = {}, {}, {}, {}
    for c in (1, 2):
        for l in range(4):
            ar_in[(c, l)] = nc.dram_tensor(f"ar_in_{c}_{l}", [128, 8], F32)
            ar_out[(c, l)] = nc.dram_tensor(f"ar_out_{c}_{l}", [128, 8], F32,
                                            addr_space="Shared")
            if l < 3:
                agp_in[(c, l)] = nc.dram_tensor(f"agp_in_{c}_{l}",
                                                [128, SB, F], F8)
                agp_out[(c, l)] = nc.dram_tensor(f"agp_out_{c}_{l}",
                                                 [NCORES * 128, SB, F], F8,
                                                 addr_space="Shared")
    ag2_in = nc.dram_tensor("ag2_in", [128, SB, F], F8)
    ag2_out = nc.dram_tensor("ag2_out", [NCORES * 128, SB, F], F8,
                             addr_space="Shared")
    warm_in = nc.dram_tensor("warm_in", [128, 8], F32)
    warm_out = nc.dram_tensor("warm_out", [128, 8], F32, addr_space="Shared")
    arg_in = nc.dram_tensor("arg_in", [128, FB * 512], F32)
    arg_out = nc.dram_tensor("arg_out", [128, FB * 512], F32,
                             addr_space="Shared")

    dbg = {}
    if os.environ.get("BASSK_DEBUG"):
        for c in (1, 2):
            for l in range(3):
                dbg[(c, l)] = nc.dram_tensor(f"dbg_p_{c}_{l}", [128, SB, F],
                                             F8, kind="ExternalOutput")
        dbg["h2q"] = nc.dram_tensor("dbg_h2q", [128, SB, F], F8,
                                    kind="ExternalOutput")

    with tile.TileContext(nc) as tc:
        with (
            tc.tile_pool(name="sb", bufs=2) as sb,
            tc.tile_pool(name="ps", bufs=4, space="PSUM") as ps,
        ):
            g = {
                "sb": sb, "ps": ps, "dbg": dbg, "partials": partials,
                "a_dram": {1: a1, 2: a2}, "mt_dram": mt,
                "p0": {1: p1_0, 2: p2_0},
                "ar_in": ar_in, "ar_out": ar_out,
                "agp_in": agp_in, "agp_out": agp_out,
                "ag2_in": ag2_in, "ag2_out": ag2_out,
                "pf": {}, "zt": {},
                "warm_in": warm_in, "warm_out": warm_out,
                "arg_in": arg_in, "arg_out": arg_out,
            }
            # ---- constants / params ----
            ident = sb.tile([128, 128], BF16, tag="ident", bufs=1, name="ident")
            make_identity(nc, ident[:])
            g["ident"] = ident
            for nm, src in (("b_sb", b_all), ("g_sb", g_all), ("bb_sb", bb_all)):
                t = sb.tile([128, 6, FB], F32, tag=nm, bufs=1, name=nm)
                nc.sync.dma_start(t[:], src.rearrange("l (m p) -> p l m", p=128))
                g[nm] = t
            al1 = sb.tile([1, 12], F32, tag="al1", bufs=1, name="al1")
            nc.sync.dma_start(al1[:], al_all[:])
            al_sb = sb.tile([128, 12], F32, tag="al_sb", bufs=1, name="al_sb")
            nc.gpsimd.partition_broadcast(al_sb[:], al1[:])
            g["al_sb"] = al_sb
            epsb = sb.tile([128, 1], F32, tag="epsb", bufs=1, name="epsb")
            nc.vector.memset(epsb[:], 1e-5)
            g["epsb"] = epsb
            ones128 = sb.tile([128, 1], BF16, tag="ones128", bufs=1, name="ones128")
            nc.vector.memset(ones128[:], 1.0)
            g["ones128"] = ones128
            onesf = sb.tile([128, 1], F32, tag="onesf", bufs=1, name="onesf")
            nc.vector.memset(onesf[:], 1.0)
            g["onesf"] = onesf
            # warm up the collective path during the startup loads
            if os.environ.get("BASSK_WARM"):
                # no measured benefit: early-collective latency jitter persists
                wsb = sb.tile([128, 8], F32, tag="wsb", bufs=1, name="wsb")
                nc.vector.memset(wsb[:], 0.0)
                nc.sync.dma_start(g["warm_in"][:], wsb[:])
                nc.gpsimd.collective_compute(
                    "AllReduce", ALU.add, replica_groups=GROUPS,
                    ins=[g["warm_in"][:]], outs=[g["warm_out"][:]])

            # ---- staggered 2-chain layer pipeline ----
            EM = STOP_LV[os.environ.get("BASSK_STOP", "full")]
            _emit_pf_load(nc, g, 1, 0)
            # ddb1 is needed by the first eviction; the rest after the A DMAs
            ddbt = {}
            t1 = sb.tile([128, SH], F32, tag="ddb1", bufs=1, name="ddb")
            nc.sync.dma_start(t1[:], ddb1[:])
            ddbt[1] = t1
            g["ddb"] = ddbt
            _emit_A_half(nc, g, 1, 0, 0)
            t2 = sb.tile([128, SH], F32, tag="ddb2", bufs=1, name="ddb")
            nc.sync.dma_start(t2[:], ddb2[:])
            ddbt[2] = t2
            dsct = {}
            for c, src in ((1, dsc1), (2, dsc2)):
                t = sb.tile([128, SB], F32, tag=f"dsc{c}", bufs=1, name="dsc")
                nc.sync.dma_start(t[:], src[:])
                dsct[c] = t
            g["dsc"] = dsct
            w_sb = sb.tile([128, 5, FB, F], BF16, tag="w_sb", bufs=1, name="w_sb")
            for li in (1, 2, 3, 4, 5):  # row 0 (enc0) applied on host
                nc.sync.dma_start(w_sb[:, li - 1], w_all[li])
            g["w_sb"] = w_sb
            at = sb.tile([128, FB, SH], BF16, tag="attrTn", bufs=1, name="attrTn")
            nc.sync.dma_start(at[:], attrTn[:])
            g["attrTn"] = at
            if EM >= 0.3:
                _emit_A_half(nc, g, 1, 0, 1)
                _emit_stats_ar(nc, g, 1, 0)
            if EM >= 0.6:
                _emit_pf_load(nc, g, 2, 0)
                _emit_A_half(nc, g, 2, 0, 0)
                _emit_bn_apply(nc, g, 1, 0)
                _emit_w_ag(nc, g, 1, 0)
            if EM >= 1:
                _emit_A_half(nc, g, 2, 0, 1)
                _emit_stats_ar(nc, g, 2, 0)
            if EM >= 2:
                for l in range(1, 4):
                    _emit_pf_load(nc, g, 1, l)
                    _emit_A_half(nc, g, 1, l, 0)
                    _emit_bn_apply(nc, g, 2, l - 1)
                    _emit_w_ag(nc, g, 2, l - 1)
                    _emit_A_half(nc, g, 1, l, 1)
                    _emit_stats_ar(nc, g, 1, l)
                    _emit_pf_load(nc, g, 2, l)
                    _emit_A_half(nc, g, 2, l, 0)
                    if l < 3:
                        _emit_bn_apply(nc, g, 1, l)
                        _emit_w_ag(nc, g, 1, l)
                        _emit_A_half(nc, g, 2, l, 1)
                        _emit_stats_ar(nc, g, 2, l)
                    else:
                        _emit_A_half(nc, g, 2, l, 1)
                        _emit_stats_ar(nc, g, 2, l)
                        _emit_bn_apply(nc, g, 1, l)
                        if EM >= 3:
                            _emit_tail1(nc, g)
            if EM >= 4:
                _emit_bn_apply(nc, g, 2, 3)
                _emit_tail2(nc, g)
            if EM >= 5:
                _emit_tail_mh_g(nc, g)
            _emit_partials(nc, g)

    nc.compile()
    return nc


_NC_CACHE = None


def _get_nc():
    global _NC_CACHE
    if _NC_CACHE is None:
        _NC_CACHE = build_nc()
    return _NC_CACHE


def _dinv(idx):
    deg = np.bincount(idx, minlength=N).astype(np.float32)
    return 1.0 / np.sqrt(np.clip(deg, 1.0, None))


def _adj_t(src, dst):
    """A^T[s, d] = multiplicity of edge s->d, float32 [N, N]."""
    flat = src.astype(np.int64) * N + dst.astype(np.int64)
    return np.bincount(flat, minlength=N * N).astype(np.float32).reshape(N, N)


def _swz_nodes(x, width):
    """[8192, width] -> [128, 64, width] with node = t*128 + p."""
    return np.ascontiguousarray(
        x.reshape(NB, 128, width).transpose(1, 0, 2))


def _swz_a(x):
    """[8192, 1024] -> [2, 128, 64, 512]: per dest-half, contiguous chunks."""
    sw = x.reshape(NB, 128, 2, 512).transpose(2, 0, 1, 3)
    return np.ascontiguousarray(sw.transpose(0, 2, 1, 3))


def host_prep(inputs):
    f8 = ml_dtypes.float8_e4m3
    bf16 = ml_dtypes.bfloat16
    attr = np.asarray(inputs["attr"], np.float32)
    matrix = np.asarray(inputs["matrix"], np.float32)
    mask1 = np.asarray(inputs["enc_mask_token1"], np.float32)
    src = np.asarray(inputs["src"]); dst = np.asarray(inputs["dst"])
    src2 = np.asarray(inputs["src2"]); dst2 = np.asarray(inputs["dst2"])
    tok = np.asarray(inputs["token_nodes"])
    noi = np.asarray(inputs["noise_nodes"])
    nsrc = np.asarray(inputs["noise_src"])

    x = attr.copy()
    x[tok] = 0.0
    x[noi] = attr[nsrc]
    np.add.at(x, tok, mask1[0])

    d1s, d1d = _dinv(src), _dinv(dst)
    d2s, d2d = _dinv(src2), _dinv(dst2)

    a1t = _adj_t(src, dst)    # A^T[s, d]
    a2t = _adj_t(src2, dst2)

    W0 = np.asarray(inputs["enc_W"][0], np.float32)
    p1_0 = _swz_nodes((d1s[:, None] * (x @ W0)).astype(f8), F)
    p2_0 = _swz_nodes((d2s[:, None] * (attr @ W0)).astype(f8), F)

    # w rows: enc0 enc1 d10 d11 d20 d21; device layout [6, 128, 4, 512]
    w_list = [np.asarray(inputs["enc_W"][0]), np.asarray(inputs["enc_W"][1]),
              np.asarray(inputs["dec1_W"][0]), np.asarray(inputs["dec1_W"][1]),
              np.asarray(inputs["dec2_W"][0]), np.asarray(inputs["dec2_W"][1])]
    w_all = np.ascontiguousarray(np.stack(
        [w.reshape(FB, 128, F).transpose(1, 0, 2) for w in w_list]
    ).astype(bf16))

    def stack6(key):
        return np.stack([
            np.asarray(inputs[f"enc_{key}"][0]), np.asarray(inputs[f"enc_{key}"][1]),
            np.asarray(inputs[f"dec1_{key}"][0]), np.asarray(inputs[f"dec1_{key}"][1]),
            np.asarray(inputs[f"dec2_{key}"][0]), np.asarray(inputs[f"dec2_{key}"][1]),
        ]).astype(np.float32)

    b_all, g_all, bb_all = stack6("b"), stack6("g"), stack6("bb")
    al = np.zeros((1, 12), np.float32)
    for i, (sa, so) in enumerate((("enc", 0), ("enc", 1), ("dec1", 0),
                                  ("dec1", 1), ("dec2", 0), ("dec2", 1))):
        al[0, 2 * i] = np.asarray(inputs[f"{sa}_ain"])[so]
        al[0, 2 * i + 1] = np.asarray(inputs[f"{sa}_aout"])[so]

    an = attr / np.maximum(np.linalg.norm(attr, axis=-1, keepdims=True), 1e-12)
    sumM2 = float(np.sum(matrix.astype(np.float64) ** 2))

    a1q = a1t.astype(f8)
    a2q = a2t.astype(f8)

    in_maps = []
    for c in range(NCORES):
        sl = slice(c * SH, (c + 1) * SH)
        # attrTn: feature-major [128, 4, 1024] for this shard
        at_sh = np.ascontiguousarray(
            an[sl].T.reshape(FB, 128, SH).transpose(1, 0, 2)).astype(bf16)
        in_maps.append({
            "p1_0": p1_0, "p2_0": p2_0,
            "a1": _swz_a(a1q[:, sl]),
            "a2": _swz_a(a2q[:, sl]),
            "mt": _swz_nodes(
                np.ascontiguousarray(matrix[sl].T).astype(f8), SH),
            "w_all": w_all, "b_all": b_all, "g_all": g_all, "bb_all": bb_all,
            "al_all": al,
            "ddb1": np.ascontiguousarray(
                np.broadcast_to(d1d[sl], (128, SH))).astype(np.float32),
            "ddb2": np.ascontiguousarray(
                np.broadcast_to(d2d[sl], (128, SH))).astype(np.float32),
            "dsc1": np.ascontiguousarray(d1s[sl].reshape(SB, 128).T),
            "dsc2": np.ascontiguousarray(d2s[sl].reshape(SB, 128).T),
            "attrTn": at_sh,
        })
    return in_maps, sumM2


def combine(results, sumM2):
    l1 = sum(float(r["partials"][0, 0]) for r in results)
    cross = sum(float(r["partials"][1, 0]) for r in results)
    gsq = np.mean([float(r["partials"][2, 0]) for r in results])
    loss1 = l1 / N
    loss2 = (sumM2 - 2.0 * cross + gsq) / (float(N) * N)
    return np.asarray(0.5 * loss1 + 0.5 * loss2, dtype=np.float32)


def run(inputs, trace=False, trace_kwargs=None):
    nc = _get_nc()
    in_maps, sumM2 = host_prep(inputs)
    res = run_bass_kernel_spmd(nc, in_maps, core_ids=list(range(NCORES)),
                               trace=trace, **(trace_kwargs or {}))
    return combine(res.results, sumM2), res


def kernel(**inputs) -> np.ndarray:
    out, _ = run(inputs, trace=False)
    return out


# revision 35
# speedup vs baseline: 1.0915x; 1.0589x over previous
"""Trainium2 Bass kernel for the GNN message-passing autoencoder problem.

Strategy (8 NeuronCores, SPMD), v2 (fp8):
  - Nodes sharded 1024/core. Message passing is a dense matmul against the
    PLAIN adjacency transpose shard A^T[:, shard] in fp8 e4m3 (counts are
    exact in fp8) using DoubleRow perf mode. GraphConv 'both' norms are
    folded into per-node scalings: D_src^-1/2 is applied to the (h @ W)
    activations (exact per-partition scale), D_dst^-1/2 multiplies the
    aggregation PSUM before bias+PReLU.
  - The per-layer linear W is applied BEFORE the AllGather (z = A (h W) ==
    (A h) W): lhsT = feature-major BN'd h, rhs = W, giving node-major
    activations p directly - no PE transposes in the layer loop. p is
    quantized to fp8 and AllGathered (4 MB full graph).
  - Layer epilogue: bias+PReLU fused in the PSUM eviction (scalar engine),
    BN stats partials AllReduced (4 KB), BN+PReLU fused in one activation.
  - The two chains are interleaved with a half-layer stagger so ARs/AGs hide
    under the other chain's matmuls.
  - Tail: loss2*N^2 = sum(M^2) - 2*tr(H^T M H) + ||H^T H||_F^2 with
    H = l2-normalized h2 in fp8. sum(M^2) on host; tr term via an fp8
    DoubleRow matmul (M^T shard stationary, gathered H moving) with a fused
    multiply-accumulate eviction; G = H^T H computed redundantly per core.
    loss1 (cosine^3) is computed per-shard in feature-major layout using
    ones-vector matmuls for the partition reductions.
"""

import os
import sys

for _p in ("/opt/trn_rl_repo", "/opt/pypackages"):
    if _p not in sys.path:
        sys.path.append(_p)

import numpy as np
import ml_dtypes

import concourse.bass as bass
import concourse.mybir as mybir
import concourse.tile as tile
from concourse import bacc
from concourse.bass_utils import run_bass_kernel_spmd
from concourse.masks import make_identity

F8 = mybir.dt.float8e4
BF16 = mybir.dt.bfloat16
F32 = mybir.dt.float32
AF = mybir.ActivationFunctionType
ALU = mybir.AluOpType
AX = mybir.AxisListType
DR = mybir.MatmulPerfMode.DoubleRow

N = 8192
F = 512
NCORES = 8
SH = N // NCORES          # 1024 nodes per core shard
NB = N // 128             # 64 node k-subtiles
SB = SH // 128            # 8 node blocks per shard
FB = F // 128             # 4 feature blocks
GROUPS = [list(range(NCORES))]

# layer-instance parameter rows: enc0 enc1 dec1_0 dec1_1 dec2_0 dec2_1
LI = {1: [0, 1, 2, 3], 2: [0, 1, 4, 5]}
# W row applied at the END of layer l (producing p for layer l+1)
WNEXT = {1: [1, 2, 3, None], 2: [1, 4, 5, None]}


def _emit_pf_load(nc, g, c, l):
    """Load the full-graph node-major fp8 activations for layer l."""
    sb = g["sb"]
    pf = sb.tile([128, NB, F], F8, tag=f"pf{c}", bufs=1, name="pf")
    if l == 0:
        src = g["p0"][c]
        nc.sync.dma_start(pf[:, 0:8, :], src[:, 0:8, :])
        if c == 1:  # startup: let A half0's first chunks jump the queue
            g["pf_rest"] = [(pf[:, 8 * q:8 * q + 8, :],
                             src[:, 8 * q:8 * q + 8, :]) for q in range(1, 8)]
        else:
            for q in range(1, 8):
                nc.sync.dma_start(pf[:, 8 * q:8 * q + 8, :],
                                  src[:, 8 * q:8 * q + 8, :])
    else:
        src = g["agp_out"][(c, l - 1)]
        for cc in range(NCORES):
            nc.sync.dma_start(pf[:, 8 * cc:8 * cc + 8, :],
                              src[cc * 128:(cc + 1) * 128, :, :])
    g["pf"][c] = pf


def _emit_A_half(nc, g, c, l, half):
    """A-aggregation matmuls for one 512-dest half; evict with bias+PReLU."""
    sb, ps = g["sb"], g["ps"]
    li = LI[c][l]
    pf = g["pf"][c]
    a_dram = g["a_dram"][c]
    if half == 0:
        zt = sb.tile([128, FB, SH], BF16, tag=f"zt{c}", bufs=1, name="zt")
        g["zt"][c] = zt
    else:
        zt = g["zt"][c]
    zps = [ps.tile([128, 512], F32, tag=f"ps{c}", bufs=4, name="zps")
           for _ in range(FB)]
    arts = {}
    if g.get("pf_rest"):
        # emit the first A chunks, then the deferred pf chunks, then (below)
        # all matmuls - every pf DMA still precedes every matmul
        for th in range(3):
            art = sb.tile([128, 4, 512], F8, tag=f"a{c}", bufs=3, name="art")
            nc.sync.dma_start(art[:], a_dram[half, :, 4 * th:4 * th + 4, :])
            arts[th] = art
        for d_, s_ in g.pop("pf_rest"):
            nc.sync.dma_start(d_, s_)
    for th in range(16):
        if th in arts:
            art = arts[th]
        else:
            art = sb.tile([128, 4, 512], F8, tag=f"a{c}", bufs=3, name="art")
            nc.sync.dma_start(art[:], a_dram[half, :, 4 * th:4 * th + 4, :])
        for j in range(2):
            kp = 2 * th + j
            kk = 4 * th + 2 * j
            for m in range(FB):
                nc.tensor.matmul(
                    zps[m][:],
                    pf[:, kk:kk + 2, m * 128:(m + 1) * 128],
                    art[:, 2 * j:2 * j + 2, :],
                    start=(kp == 0), stop=(kp == 31), perf_mode=DR)
    for m in range(FB):
        dst = zt[:, m, half * 512:(half + 1) * 512]
        bias = g["b_sb"][:, li, m:m + 1]
        alpha = g["al_sb"][:, 2 * li:2 * li + 1]
        if l < 2:  # enc layer: multiply by ddst before bias+prelu
            zsc = sb.tile([128, 512], F32, tag="scrh", bufs=4, name="zsc")
            nc.vector.tensor_tensor(
                zsc[:], zps[m][:],
                g["ddb"][c][:, half * 512:(half + 1) * 512], ALU.mult)
            nc.scalar.activation(dst, zsc[:], AF.Prelu, bias=bias, scale=1.0,
                                 alpha=alpha)
        else:
            nc.scalar.activation(dst, zps[m][:], AF.Prelu, bias=bias,
                                 scale=1.0, alpha=alpha)


def _emit_stats_ar(nc, g, c, l):
    """Per-core BN stats (sum, sumsq per feature) and the AllReduce."""
    sb = g["sb"]
    zt = g["zt"][c]
    stats = sb.tile([128, 8], F32, tag=f"st{c}", bufs=1, name="stats")
    for m in range(FB):
        nc.vector.reduce_sum(stats[:, 2 * m:2 * m + 1], zt[:, m, :], axis=AX.X)
        scr = sb.tile([128, SH], F32, tag="scr", bufs=1, name="scr")
        nc.scalar.activation(scr[:], zt[:, m, :], AF.Square,
                             accum_out=stats[:, 2 * m + 1:2 * m + 2])
    ar_in = g["ar_in"][(c, l)]
    ar_out = g["ar_out"][(c, l)]
    nc.sync.dma_start(ar_in[:], stats[:])
    nc.gpsimd.collective_compute(
        "AllReduce", ALU.add, replica_groups=GROUPS,
        ins=[ar_in[:]], outs=[ar_out[:]])


def _emit_bn_apply(nc, g, c, l):
    """BN finalize from the AllReduced stats; fused BN+PReLU in place."""
    sb = g["sb"]
    li = LI[c][l]
    zt = g["zt"][c]
    gstats = sb.tile([128, 8], F32, tag="gstats", name="gstats")
    nc.sync.dma_start(gstats[:], g["ar_out"][(c, l)][:])
    mean = sb.tile([128, FB], F32, tag="mean", name="mean")
    var = sb.tile([128, FB], F32, tag="var", name="var")
    sN = sb.tile([128, FB], F32, tag="sN", name="sN")
    tN = sb.tile([128, FB], F32, tag="tN", name="tN")
    m2 = sb.tile([128, FB], F32, tag="m2", name="m2")
    nc.scalar.mul(mean[:], gstats[:, 0:8:2], 1.0 / N)
    nc.scalar.mul(var[:], gstats[:, 1:8:2], 1.0 / N)      # E[x^2]
    nc.vector.tensor_mul(m2[:], mean[:], mean[:])
    nc.vector.tensor_sub(var[:], var[:], m2[:])
    nc.scalar.activation(sN[:], var[:], AF.Sqrt, bias=g["epsb"][:])
    nc.vector.reciprocal(sN[:], sN[:])
    nc.vector.tensor_mul(sN[:], sN[:], g["g_sb"][:, li, :])
    nc.vector.tensor_mul(m2[:], mean[:], sN[:])
    nc.vector.tensor_sub(tN[:], g["bb_sb"][:, li, :], m2[:])
    for m in range(FB):
        nc.scalar.activation(
            zt[:, m, :], zt[:, m, :], AF.Prelu,
            bias=tN[:, m:m + 1], scale=sN[:, m:m + 1],
            alpha=g["al_sb"][:, 2 * li + 1:2 * li + 2])


def _emit_w_ag(nc, g, c, l):
    """p = (BN'd h) @ W_next (node-major out), fp8 quantize, AllGather."""
    sb, ps = g["sb"], g["ps"]
    zt = g["zt"][c]
    li_w = WNEXT[c][l]
    p_out = sb.tile([128, SB, F], F8, tag=f"po{c}", bufs=2, name="p_out")
    for wave in range(2):
        pps = [ps.tile([128, 512], F32, tag=f"ps{c}", bufs=4, name="pps")
               for _ in range(4)]
        for i in range(4):
            tb = wave * 4 + i
            for kb in range(FB):
                nc.tensor.matmul(
                    pps[i][:], zt[:, kb, tb * 128:(tb + 1) * 128],
                    g["w_sb"][:, li_w - 1, kb, :],
                    start=(kb == 0), stop=(kb == FB - 1))
        for i in range(4):
            tb = wave * 4 + i
            if l == 0:  # next layer is enc (normalized): scale by dsrc
                nc.vector.tensor_scalar_mul(
                    p_out[:, tb, :], pps[i][:], g["dsc"][c][:, tb:tb + 1])
            else:
                nc.vector.tensor_copy(p_out[:, tb, :], pps[i][:])
    agp_in = g["agp_in"][(c, l)]
    nc.sync.dma_start(agp_in[:], p_out[:])
    nc.gpsimd.collective_compute(
        "AllGather", ALU.bypass, replica_groups=GROUPS,
        ins=[agp_in[:]], outs=[g["agp_out"][(c, l)][:]])
    if (c, l) in g["dbg"]:
        nc.sync.dma_start(g["dbg"][(c, l)][:], p_out[:])


def _emit_tail1(nc, g):
    """Chain1 tail: loss1 partial = sum over shard of (1 - cos(h1, attr))^3."""
    sb, ps = g["sb"], g["ps"]
    zt = g["zt"][1]
    attrTn = g["attrTn"]
    dotps = [ps.tile([1, 512], F32, tag="ps1", bufs=4, name="dotps")
             for _ in range(2)]
    n1ps = [ps.tile([1, 512], F32, tag="ps1", bufs=4, name="n1ps")
            for _ in range(2)]
    for kb in range(FB):
        prod = sb.tile([128, SH], BF16, tag="prod", bufs=2, name="prod")
        nc.vector.tensor_tensor(prod[:], zt[:, kb, :], attrTn[:, kb, :],
                                ALU.mult)
        sq = sb.tile([128, SH], BF16, tag="sqh", bufs=2, name="sq")
        nc.gpsimd.tensor_tensor(sq[:], zt[:, kb, :], zt[:, kb, :], ALU.mult)
        for h in range(2):
            nc.tensor.matmul(dotps[h][:], g["ones128"][:],
                             prod[:, h * 512:(h + 1) * 512],
                             start=(kb == 0), stop=(kb == FB - 1))
            nc.tensor.matmul(n1ps[h][:], g["ones128"][:],
                             sq[:, h * 512:(h + 1) * 512],
                             start=(kb == 0), stop=(kb == FB - 1))
    dot_sb = sb.tile([1, SH], F32, tag="row", bufs=3, name="dot_sb")
    n1_sb = sb.tile([1, SH], F32, tag="row", bufs=3, name="n1_sb")
    for h in range(2):
        nc.vector.tensor_copy(dot_sb[:, h * 512:(h + 1) * 512], dotps[h][:])
        nc.vector.tensor_copy(n1_sb[:, h * 512:(h + 1) * 512], n1ps[h][:])
    nc.scalar.activation(n1_sb[:], n1_sb[:], AF.Sqrt)
    nc.vector.tensor_scalar_max(n1_sb[:], n1_sb[:], 1e-12)
    nc.vector.reciprocal(n1_sb[:], n1_sb[:])
    nc.vector.tensor_mul(dot_sb[:], dot_sb[:], n1_sb[:])        # cos
    u = sb.tile([1, SH], F32, tag="row", bufs=3, name="u")
    nc.scalar.activation(u[:], dot_sb[:], AF.Copy, scale=-1.0, bias=1.0)
    u2 = sb.tile([1, SH], F32, tag="row", bufs=3, name="u2")
    nc.vector.tensor_mul(u2[:], u[:], u[:])
    nc.vector.tensor_mul(u2[:], u2[:], u[:])                    # u^3
    l1p = sb.tile([1, 1], F32, tag="l1p", name="l1p")
    nc.vector.reduce_sum(l1p[:], u2[:], axis=AX.X)
    g["l1p"] = l1p


def _emit_tail2(nc, g):
    """Chain2 tail: l2-normalize h2 (via PE transposes), fp8, AllGather."""
    sb, ps = g["sb"], g["ps"]
    zt = g["zt"][2]
    h2nm = sb.tile([128, SB, F], BF16, tag="h2nm", bufs=1, name="h2nm")
    nrm2 = sb.tile([128, SB], F32, tag="nrm2", bufs=1, name="nrm2")
    for tb in range(SB):
        for m in range(FB):
            tp = ps.tile([128, 128], BF16, tag="ps2", bufs=4, name="tp")
            nc.tensor.transpose(tp[:], zt[:, m, tb * 128:(tb + 1) * 128],
                                g["ident"][:])
            nc.vector.tensor_copy(h2nm[:, tb, m * 128:(m + 1) * 128], tp[:])
        scr = sb.tile([128, 512], F32, tag="scrh", bufs=4, name="sqs")
        nc.scalar.activation(scr[:], h2nm[:, tb, :], AF.Square,
                             accum_out=nrm2[:, tb:tb + 1])
    nc.scalar.activation(nrm2[:], nrm2[:], AF.Sqrt)
    nc.vector.tensor_scalar_max(nrm2[:], nrm2[:], 1e-12)
    nc.vector.reciprocal(nrm2[:], nrm2[:])
    h2q = sb.tile([128, SB, F], F8, tag="h2q", bufs=1, name="h2q")
    for tb in range(SB):
        nc.vector.tensor_scalar_mul(h2q[:, tb, :], h2nm[:, tb, :],
                                    nrm2[:, tb:tb + 1])
    nc.sync.dma_start(g["ag2_in"][:], h2q[:])
    nc.gpsimd.collective_compute(
        "AllGather", ALU.bypass, replica_groups=GROUPS,
        ins=[g["ag2_in"][:]], outs=[g["ag2_out"][:]])
    g["h2q"] = h2q
    # local partial of G = H^T H (k over the local 8 node blocks), then
    # AllReduce it while the MH matmul runs
    gq = [ps.tile([128, 512], F32, tag="ps1", bufs=4, name="gq")
          for _ in range(FB)]
    for t in range(SB // 2):
        for mb in range(FB):
            nc.tensor.matmul(
                gq[mb][:], h2q[:, 2 * t:2 * t + 2, mb * 128:(mb + 1) * 128],
                h2q[:, 2 * t:2 * t + 2, :],
                start=(t == 0), stop=(t == SB // 2 - 1), perf_mode=DR)
    for mb in range(FB):
        gsc = sb.tile([128, 512], F32, tag="scrh", bufs=4, name="gsc")
        nc.vector.tensor_copy(gsc[:], gq[mb][:])
        nc.sync.dma_start(g["arg_in"][:, mb * 512:(mb + 1) * 512], gsc[:])
    nc.gpsimd.collective_compute(
        "AllReduce", ALU.add, replica_groups=GROUPS,
        ins=[g["arg_in"][:]], outs=[g["arg_out"][:]])
    if "h2q" in g["dbg"]:
        nc.sync.dma_start(g["dbg"]["h2q"][:], h2q[:])


def _emit_tail_mh_g(nc, g):
    """MH = M @ H (fp8 DR) with fused (MH*H) accumulation; partials."""
    sb, ps = g["sb"], g["ps"]
    # prefetch the first M^T chunks before the H reload
    mtcs = {}
    for t in range(2):
        mtc = sb.tile([128, 2, SH], F8, tag="mtc", bufs=3, name="mtc")
        nc.sync.dma_start(mtc[:], g["mt_dram"][:, 2 * t:2 * t + 2, :])
        mtcs[t] = mtc
    hf = sb.tile([128, NB, F], F8, tag="pf2", bufs=1, name="hf")
    for cc in range(NCORES):
        nc.sync.dma_start(hf[:, 8 * cc:8 * cc + 8, :],
                          g["ag2_out"][cc * 128:(cc + 1) * 128, :, :])
    mhps = [ps.tile([128, 512], F32, tag="ps1", bufs=4, name="mhps")
            for _ in range(4)]
    mhps += [ps.tile([128, 512], F32, tag="ps2", bufs=4, name="mhps2")
             for _ in range(4)]
    for t in range(32):
        if t in mtcs:
            mtc = mtcs[t]
        else:
            mtc = sb.tile([128, 2, SH], F8, tag="mtc", bufs=3, name="mtc")
            nc.sync.dma_start(mtc[:], g["mt_dram"][:, 2 * t:2 * t + 2, :])
        for ib in range(8):
            nc.tensor.matmul(
                mhps[ib][:], mtc[:, :, ib * 128:(ib + 1) * 128],
                hf[:, 2 * t:2 * t + 2, :],
                start=(t == 0), stop=(t == 31), perf_mode=DR)
    xacc = sb.tile([128, 8], F32, tag="xacc", bufs=1, name="xacc")
    h2q = g["h2q"]
    for ib in range(8):
        scr = sb.tile([128, 512], F32, tag="scrh", bufs=4, name="xscr")
        nc.vector.scalar_tensor_tensor(
            scr[:], mhps[ib][:], 1.0, h2q[:, ib, :], op0=ALU.mult,
            op1=ALU.mult, accum_out=xacc[:, ib:ib + 1])
    # gsq from the AllReduced G partials (reuses the h2nm slot)
    gsb = sb.tile([128, FB, 512], F32, tag="h2nm", bufs=1, name="gsb")
    nc.sync.dma_start(gsb[:], g["arg_out"].rearrange("p (m f) -> p m f", m=FB))
    gacc = sb.tile([128, FB], F32, tag="gacc", bufs=1, name="gacc")
    for mb in range(FB):
        scr = sb.tile([128, 512], F32, tag="scrh", bufs=4, name="gscr")
        nc.scalar.activation(scr[:], gsb[:, mb, :], AF.Square,
                             accum_out=gacc[:, mb:mb + 1])
    g["xacc"] = xacc
    g["gacc"] = gacc


def _emit_partials(nc, g):
    sb, ps = g["sb"], g["ps"]
    # combine partials: [l1p_sum, cross_sum, gsq_sum]
    pl = sb.tile([128, 3], F32, tag="pl", name="pl")
    nc.vector.memset(pl[:], 0.0)
    if "l1p" in g:
        nc.vector.tensor_copy(pl[0:1, 0:1], g["l1p"][:])
    if "xacc" in g:
        nc.vector.reduce_sum(pl[:, 1:2], g["xacc"][:], axis=AX.X)
        nc.vector.reduce_sum(pl[:, 2:3], g["gacc"][:], axis=AX.X)
    pp = ps.tile([3, 1], F32, tag="ps1", bufs=4, name="pp")
    nc.tensor.matmul(pp[:], pl[:], g["onesf"][:], start=True, stop=True)
    out_sb = sb.tile([3, 1], F32, tag="out_sb", name="out_sb")
    nc.scalar.copy(out_sb[:], pp[:])
    nc.sync.dma_start(g["partials"][:], out_sb[:])


STOP_LV = {"l0h": 0, "l0s": 0.3, "l0w": 0.6, "l0": 1, "layers": 2, "tail1": 3,
           "tail2": 4, "full": 5}


def build_nc():
    nc = bacc.Bacc("TRN2", target_bir_lowering=False, debug=False,
                   num_devices=NCORES)

    ins = {}

    def di(name, shape, dt):
        ins[name] = nc.dram_tensor(name, shape, dt, kind="ExternalInput")
        return ins[name]

    p1_0 = di("p1_0", [128, NB, F], F8)
    p2_0 = di("p2_0", [128, NB, F], F8)
    a1 = di("a1", [2, 128, NB, 512], F8)
    a2 = di("a2", [2, 128, NB, 512], F8)
    mt = di("mt", [128, NB, SH], F8)
    w_all = di("w_all", [6, 128, FB, F], BF16)
    b_all = di("b_all", [6, F], F32)
    g_all = di("g_all", [6, F], F32)
    bb_all = di("bb_all", [6, F], F32)
    al_all = di("al_all", [1, 12], F32)
    ddb1 = di("ddb1", [128, SH], F32)
    ddb2 = di("ddb2", [128, SH], F32)
    dsc1 = di("dsc1", [128, SB], F32)
    dsc2 = di("dsc2", [128, SB], F32)
    attrTn = di("attrTn", [128, FB, SH], BF16)

    partials = nc.dram_tensor("partials", [3, 1], F32, kind="ExternalOutput")

    ar_in, ar_out, agp_in, agp_out = {}, {}, {}, {}
    for c in (1, 2):
        for l in range(4):
            ar_in[(c, l)] = nc.dram_tensor(f"ar_in_{c}_{l}", [128, 8], F32)
            ar_out[(c, l)] = nc.dram_tensor(f"ar_out_{c}_{l}", [128, 8], F32,
                                            addr_space="Shared")
            if l < 3:
                agp_in[(c, l)] = nc.dram_tensor(f"agp_in_{c}_{l}",
                                                [128, SB, F], F8)
                agp_out[(c, l)] = nc.dram_tensor(f"agp_out_{c}_{l}",
                                                 [NCORES * 128, SB, F], F8,
                                                 addr_space="Shared")
    ag2_in = nc.dram_tensor("ag2_in", [128, SB, F], F8)
    ag2_out = nc.dram_tensor("ag2_out", [NCORES * 128, SB, F], F8,
                             addr_space="Shared")
    warm_in = nc.dram_tensor("warm_in", [128, 8], F32)
    warm_out = nc.dram_tensor("warm_out", [128, 8], F32, addr_space="Shared")
    arg_in = nc.dram_tensor("arg_in", [128, FB * 512], F32)
    arg_out = nc.dram_tensor("arg_out", [128, FB * 512], F32,
                             addr_space="Shared")

    dbg = {}
    if os.environ.get("BASSK_DEBUG"):
        for c in (1, 2):
            for l in range(3):
                dbg[(c, l)] = nc.dram_tensor(f"dbg_p_{c}_{l}", [128, SB, F],
                                             F8, kind="ExternalOutput")
        dbg["h2q"] = nc.dram_tensor("dbg_h2q", [128, SB, F], F8,
                                    kind="ExternalOutput")

    with tile.TileContext(nc) as tc:
        with (
            tc.tile_pool(name="sb", bufs=2) as sb,
            tc.tile_pool(name="ps", bufs=4, space="PSUM") as ps,
        ):
            g = {
                "sb": sb, "ps": ps, "dbg": dbg, "partials": partials,
                "a_dram": {1: a1, 2: a2}, "mt_dram": mt,
                "p0": {1: p1_0, 2: p2_0},
                "ar_in": ar_in, "ar_out": ar_out,
                "agp_in": agp_in, "agp_out": agp_out,
                "ag2_in": ag2_in, "ag2_out": ag2_out,
                "pf": {}, "zt": {},
                "warm_in": warm_in, "warm_out": warm_out,
                "arg_in": arg_in, "arg_out": arg_out,
            }
            # ---- constants / params ----
            ident = sb.tile([128, 128], BF16, tag="ident", bufs=1, name="ident")
            make_identity(nc, ident[:])
            g["ident"] = ident
            for nm, src in (("b_sb", b_all), ("g_sb", g_all), ("bb_sb", bb_all)):
                t = sb.tile([128, 6, FB], F32, tag=nm, bufs=1, name=nm)
                nc.sync.dma_start(t[:], src.rearrange("l (m p) -> p l m", p=128))
                g[nm] = t
            al1 = sb.tile([1, 12], F32, tag="al1", bufs=1, name="al1")
            nc.sync.dma_start(al1[:], al_all[:])
            al_sb = sb.tile([128, 12], F32, tag="al_sb", bufs=1, name="al_sb")
            nc.gpsimd.partition_broadcast(al_sb[:], al1[:])
            g["al_sb"] = al_sb
            epsb = sb.tile([128, 1], F32, tag="epsb", bufs=1, name="epsb")
            nc.vector.memset(epsb[:], 1e-5)
            g["epsb"] = epsb
            ones128 = sb.tile([128, 1], BF16, tag="ones128", bufs=1, name="ones128")
            nc.vector.memset(ones128[:], 1.0)
            g["ones128"] = ones128
            onesf = sb.tile([128, 1], F32, tag="onesf", bufs=1, name="onesf")
            nc.vector.memset(onesf[:], 1.0)
            g["onesf"] = onesf
            # warm up the collective path during the startup loads
            if os.environ.get("BASSK_WARM"):
                # no measured benefit: early-collective latency jitter persists
                wsb = sb.tile([128, 8], F32, tag="wsb", bufs=1, name="wsb")
                nc.vector.memset(wsb[:], 0.0)
                nc.sync.dma_start(g["warm_in"][:], wsb[:])
                nc.gpsimd.collective_compute(
                    "AllReduce", ALU.add, replica_groups=GROUPS,
                    ins=[g["warm_in"][:]], outs=[g["warm_out"][:]])

            # ---- staggered 2-chain layer pipeline ----
            EM = STOP_LV[os.environ.get("BASSK_STOP", "full")]
            _emit_pf_load(nc, g, 1, 0)
            # ddb1 is needed by the first eviction; the rest after the A DMAs
            ddbt = {}
            t1 = sb.tile([128, SH], F32, tag="ddb1", bufs=1, name="ddb")
            nc.sync.dma_start(t1[:], ddb1[:])
            ddbt[1] = t1
            g["ddb"] = ddbt
            _emit_A_half(nc, g, 1, 0, 0)
            t2 = sb.tile([128, SH], F32, tag="ddb2", bufs=1, name="ddb")
            nc.sync.dma_start(t2[:], ddb2[:])
            ddbt[2] = t2
            dsct = {}
            for c, src in ((1, dsc1), (2, dsc2)):
                t = sb.tile([128, SB], F32, tag=f"dsc{c}", bufs=1, name="dsc")
                nc.sync.dma_start(t[:], src[:])
                dsct[c] = t
            g["dsc"] = dsct
            w_sb = sb.tile([128, 5, FB, F], BF16, tag="w_sb", bufs=1, name="w_sb")
            for li in (1, 2, 3, 4, 5):  # row 0 (enc0) applied on host
                nc.sync.dma_start(w_sb[:, li - 1], w_all[li])
            g["w_sb"] = w_sb
            at = sb.tile([128, FB, SH], BF16, tag="attrTn", bufs=1, name="attrTn")
            nc.sync.dma_start(at[:], attrTn[:])
            g["attrTn"] = at
            if EM >= 0.3:
                _emit_A_half(nc, g, 1, 0, 1)
                _emit_stats_ar(nc, g, 1, 0)
            if EM >= 0.6:
                _emit_pf_load(nc, g, 2, 0)
                _emit_A_half(nc, g, 2, 0, 0)
                _emit_bn_apply(nc, g, 1, 0)
                _emit_w_ag(nc, g, 1, 0)
            if EM >= 1:
                _emit_A_half(nc, g, 2, 0, 1)
                _emit_stats_ar(nc, g, 2, 0)
            if EM >= 2:
                for l in range(1, 4):
                    _emit_pf_load(nc, g, 1, l)
                    _emit_A_half(nc, g, 1, l, 0)
                    _emit_bn_apply(nc, g, 2, l - 1)
                    _emit_w_ag(nc, g, 2, l - 1)
                    _emit_A_half(nc, g, 1, l, 1)
                    _emit_stats_ar(nc, g, 1, l)
                    _emit_pf_load(nc, g, 2, l)
                    _emit_A_half(nc, g, 2, l, 0)
                    if l < 3:
                        _emit_bn_apply(nc, g, 1, l)
                        _emit_w_ag(nc, g, 1, l)
                        _emit_A_half(nc, g, 2, l, 1)
                        _emit_stats_ar(nc, g, 2, l)
                    else:
                        _emit_A_half(nc, g, 2, l, 1)
                        _emit_stats_ar(nc, g, 2, l)
                        _emit_bn_apply(nc, g, 1, l)
                        if EM >= 3:
                            _emit_tail1(nc, g)
            if EM >= 4:
                _emit_bn_apply(nc, g, 2, 3)
                _emit_tail2(nc, g)
            if EM >= 5:
                _emit_tail_mh_g(nc, g)
            _emit_partials(nc, g)

    nc.compile()
    return nc


_NC_CACHE = None


def _get_nc():
    global _NC_CACHE
    if _NC_CACHE is None:
        _NC_CACHE = build_nc()
    return _NC_CACHE


def _dinv(idx):
    deg = np.bincount(idx, minlength=N).astype(np.float32)
    return 1.0 / np.sqrt(np.clip(deg, 1.0, None))


def _adj_t(src, dst):
    """A^T[s, d] = multiplicity of edge s->d, float32 [N, N]."""
    flat = src.astype(np.int64) * N + dst.astype(np.int64)
    return np.bincount(flat, minlength=N * N).astype(np.float32).reshape(N, N)


def _swz_nodes(x, width):
    """[8192, width] -> [128, 64, width] with node = t*128 + p."""
    return np.ascontiguousarray(
        x.reshape(NB, 128, width).transpose(1, 0, 2))


def _swz_a(x):
    """[8192, 1024] -> [2, 128, 64, 512]: per dest-half, contiguous chunks."""
    sw = x.reshape(NB, 128, 2, 512).transpose(2, 0, 1, 3)
    return np.ascontiguousarray(sw.transpose(0, 2, 1, 3))


def host_prep(inputs):
    f8 = ml_dtypes.float8_e4m3
    bf16 = ml_dtypes.bfloat16
    attr = np.asarray(inputs["attr"], np.float32)
    matrix = np.asarray(inputs["matrix"], np.float32)
    mask1 = np.asarray(inputs["enc_mask_token1"], np.float32)
    src = np.asarray(inputs["src"]); dst = np.asarray(inputs["dst"])
    src2 = np.asarray(inputs["src2"]); dst2 = np.asarray(inputs["dst2"])
    tok = np.asarray(inputs["token_nodes"])
    noi = np.asarray(inputs["noise_nodes"])
    nsrc = np.asarray(inputs["noise_src"])

    x = attr.copy()
    x[tok] = 0.0
    x[noi] = attr[nsrc]
    np.add.at(x, tok, mask1[0])

    d1s, d1d = _dinv(src), _dinv(dst)
    d2s, d2d = _dinv(src2), _dinv(dst2)

    a1t = _adj_t(src, dst)    # A^T[s, d]
    a2t = _adj_t(src2, dst2)

    W0 = np.asarray(inputs["enc_W"][0], np.float32)
    p1_0 = _swz_nodes((d1s[:, None] * (x @ W0)).astype(f8), F)
    p2_0 = _swz_nodes((d2s[:, None] * (attr @ W0)).astype(f8), F)

    # w rows: enc0 enc1 d10 d11 d20 d21; device layout [6, 128, 4, 512]
    w_list = [np.asarray(inputs["enc_W"][0]), np.asarray(inputs["enc_W"][1]),
              np.asarray(inputs["dec1_W"][0]), np.asarray(inputs["dec1_W"][1]),
              np.asarray(inputs["dec2_W"][0]), np.asarray(inputs["dec2_W"][1])]
    w_all = np.ascontiguousarray(np.stack(
        [w.reshape(FB, 128, F).transpose(1, 0, 2) for w in w_list]
    ).astype(bf16))

    def stack6(key):
        return np.stack([
            np.asarray(inputs[f"enc_{key}"][0]), np.asarray(inputs[f"enc_{key}"][1]),
            np.asarray(inputs[f"dec1_{key}"][0]), np.asarray(inputs[f"dec1_{key}"][1]),
            np.asarray(inputs[f"dec2_{key}"][0]), np.asarray(inputs[f"dec2_{key}"][1]),
        ]).astype(np.float32)

    b_all, g_all, bb_all = stack6("b"), stack6("g"), stack6("bb")
    al = np.zeros((1, 12), np.float32)
    for i, (sa, so) in enumerate((("enc", 0), ("enc", 1), ("dec1", 0),
                                  ("dec1", 1), ("dec2", 0), ("dec2", 1))):
        al[0, 2 * i] = np.asarray(inputs[f"{sa}_ain"])[so]
        al[0, 2 * i + 1] = np.asarray(inputs[f"{sa}_aout"])[so]

    an = attr / np.maximum(np.linalg.norm(attr, axis=-1, keepdims=True), 1e-12)
    sumM2 = float(np.sum(matrix.astype(np.float64) ** 2))

    a1q = a1t.astype(f8)
    a2q = a2t.astype(f8)

    in_maps = []
    for c in range(NCORES):
        sl = slice(c * SH, (c + 1) * SH)
        # attrTn: feature-major [128, 4, 1024] for this shard
        at_sh = np.ascontiguousarray(
            an[sl].T.reshape(FB, 128, SH).transpose(1, 0, 2)).astype(bf16)
        in_maps.append({
            "p1_0": p1_0, "p2_0": p2_0,
            "a1": _swz_a(a1q[:, sl]),
            "a2": _swz_a(a2q[:, sl]),
            "mt": _swz_nodes(
                np.ascontiguousarray(matrix[sl].T).astype(f8), SH),
            "w_all": w_all, "b_all": b_all, "g_all": g_all, "bb_all": bb_all,
            "al_all": al,
            "ddb1": np.ascontiguousarray(
                np.broadcast_to(d1d[sl], (128, SH))).astype(np.float32),
            "ddb2": np.ascontiguousarray(
                np.broadcast_to(d2d[sl], (128, SH))).astype(np.float32),
            "dsc1": np.ascontiguousarray(d1s[sl].reshape(SB, 128).T),
            "dsc2": np.ascontiguousarray(d2s[sl].reshape(SB, 128).T),
            "attrTn": at_sh,
        })
    return in_maps, sumM2


def combine(results, sumM2):
    l1 = sum(float(r["partials"][0, 0]) for r in results)
    cross = sum(float(r["partials"][1, 0]) for r in results)
    gsq = np.mean([float(r["partials"][2, 0]) for r in results])
    loss1 = l1 / N
    loss2 = (sumM2 - 2.0 * cross + gsq) / (float(N) * N)
    return np.asarray(0.5 * loss1 + 0.5 * loss2, dtype=np.float32)


def run(inputs, trace=False, trace_kwargs=None):
    nc = _get_nc()
    in_maps, sumM2 = host_prep(inputs)
    res = run_bass_kernel_spmd(nc, in_maps, core_ids=list(range(NCORES)),
                               trace=trace, **(trace_kwargs or {}))
    return combine(res.results, sumM2), res


def kernel(**inputs) -> np.ndarray:
    out, _ = run(inputs, trace=False)
    return out
